# revision 1
# baseline (speedup 1.0000x reference)
"""Trainium2 Bass kernel for nn_MultiHeadAttention (channel-attention transformer block).

Math (per batch b, with X* = reshape(*, [C, P]), P = 4096, C = 128, D = 512):
  Q = Xq @ (Wq/temp)^T, K = Xk @ Wk^T, V = Xv @ Wv^T            [C, D]
  per head h (8 heads, ld=64): A_h = softmax(Q_h K_h^T); O_h = A_h V_h
  O = silu(O); O = (O - mean)/(unbiased_std + eps)   (LN affine folded into fc)
  out_pre = (v + Wfc@ln_beta) + O @ (Wfc*ln_gamma)^T
  out = BatchNorm2d(out_pre)   (batch stats over (b,h,w), biased var)

Sharding: data-parallel over batch, 2 batches per core on 8 cores; BatchNorm
statistics combined with a tiny AllReduce ([128,2] per core).

Matmul dtype: float32r (fp32 bits, full PE rate at N>=256); BASS_MM_MODE can
switch to bf16 or plain f32. All inputs are host-packed so every DMA is a
fully contiguous transfer on both DRAM and SBUF sides.
"""

import os

import numpy as np

import concourse.mybir as mybir
import concourse.tile as tile
from concourse import bacc
from concourse.bass_utils import run_bass_kernel_spmd
from concourse.masks import make_identity

# ---- problem constants (hardcoded per contract) ----
B, C, HH, WW = 16, 128, 64, 64
P = HH * WW           # 4096
NH, LD = 8, 64
D = NH * LD           # 512
N_CORES = 8
BPC = B // N_CORES    # 2 batches per core
NPC = P // 512        # 8 quad-chunks over contraction / output tiles
LN_EPS = 1e-6
BN_EPS = 1e-5
F32 = mybir.dt.float32
F32R = mybir.dt.float32r
BF16 = mybir.dt.bfloat16

MODE = os.environ.get("BASS_MM_MODE", "f32r")  # f32r | bf16 | f32

_BUILD_CACHE: dict = {}
LAST_RESULTS = None  # BassKernelResults of the most recent run (for profiling)


def _emit(ctx, nc, tc, io):
    act_dt = {"f32r": F32R, "bf16": BF16, "f32": F32}[MODE]
    AF = mybir.ActivationFunctionType
    ALU = mybir.AluOpType
    AX = mybir.AxisListType

    def raw(ap):
        # f32 view of an f32r AP for DVE reads (pure byte copy, no re-round)
        return ap.bitcast(F32) if MODE == "f32r" else ap

    consts = ctx.enter_context(tc.tile_pool(name="consts", bufs=1))
    wpool = ctx.enter_context(tc.tile_pool(name="wpool", bufs=2))
    fcpool = ctx.enter_context(tc.tile_pool(name="fcpool", bufs=5))
    apool = ctx.enter_context(tc.tile_pool(name="apool", bufs=2))
    big = ctx.enter_context(tc.tile_pool(name="big", bufs=1))
    sb = ctx.enter_context(tc.tile_pool(name="sb", bufs=2))
    small = ctx.enter_context(tc.tile_pool(name="small", bufs=4))
    stat = ctx.enter_context(tc.tile_pool(name="stat", bufs=1))
    dram = ctx.enter_context(tc.tile_pool(name="dram", bufs=1, space="DRAM"))

    # identity for PE transposes (made in f32, cast to the matmul dtype);
    # a dummy transpose primes PE's view of the identity writer so later
    # transposes carry a single sync wait (HW allows 1 per instruction)
    ident_f = consts.tile([128, 128], F32, tag="identf", name="identf")
    make_identity(nc, ident_f)
    if MODE == "f32":
        ident = ident_f
    else:
        ident = consts.tile([128, 128], act_dt, tag="ident", name="ident")
        nc.vector.tensor_copy(out=ident, in_=ident_f)

    bng = consts.tile([128, 1], F32, tag="bng", name="bng")
    bnb = consts.tile([128, 1], F32, tag="bnb", name="bnb")
    nc.gpsimd.dma_start(out=bng, in_=io["bng"][:, :])
    nc.gpsimd.dma_start(out=bnb, in_=io["bnb"][:, :])

    out_sb = []
    for b in range(BPC):
        t = big.tile([128, P], F32, tag=f"veff{b}", name=f"veff{b}")
        nc.gpsimd.dma_start(out=t, in_=io["veff"][b, :, :])
        out_sb.append(t)

    # ---- phase A: QKV projections, accumulating over the P=4096 contraction ----
    ps_proj = ctx_a = tc.tile_pool(name="ps_proj", bufs=1, space="PSUM")
    ps_proj = ctx_a.__enter__()
    warm = ps_proj.tile([128, 128], act_dt, tag="warm", name="warm")
    nc.tensor.transpose(warm[:, :], ident[:, :], ident[:, :])
    Qp = [ps_proj.tile([128, D], F32, tag=f"Qp{b}", name=f"Qp{b}") for b in range(BPC)]
    Kp = [ps_proj.tile([128, D], F32, tag=f"Kp{b}", name=f"Kp{b}") for b in range(BPC)]
    Vp = [ps_proj.tile([128, D], F32, tag=f"Vp{b}", name=f"Vp{b}") for b in range(BPC)]

    for pc in range(NPC):
        wq_c = wpool.tile([128, 4, D], act_dt, tag="wq_c", name="wq_c")
        wk_c = wpool.tile([128, 4, D], act_dt, tag="wk_c", name="wk_c")
        wv_c = wpool.tile([128, 4, D], act_dt, tag="wv_c", name="wv_c")
        nc.sync.dma_start(out=wq_c, in_=io["wq"][pc])
        nc.scalar.dma_start(out=wk_c, in_=io["wk"][pc])
        nc.gpsimd.dma_start(out=wv_c, in_=io["wv"][pc])
        qcs, kcs, vcs = [], [], []
        for b in range(BPC):
            qc = apool.tile([128, 4, 128], act_dt, tag=f"qc{b}", name=f"qc{b}")
            kc = apool.tile([128, 4, 128], act_dt, tag=f"kc{b}", name=f"kc{b}")
            vc = apool.tile([128, 4, 128], act_dt, tag=f"vc{b}", name=f"vc{b}")
            nc.sync.dma_start(out=qc, in_=io["qT"][b, pc])
            nc.scalar.dma_start(out=kc, in_=io["kT"][b, pc])
            nc.gpsimd.dma_start(out=vc, in_=io["vT"][b, pc])
            qcs.append(qc); kcs.append(kc); vcs.append(vc)
        for j in range(4):
            st = pc == 0 and j == 0
            sp = pc == NPC - 1 and j == 3
            for b in range(BPC):
                nc.tensor.matmul(Qp[b][:, :], qcs[b][:, j, :], wq_c[:, j, :], start=st, stop=sp)
                nc.tensor.matmul(Kp[b][:, :], kcs[b][:, j, :], wk_c[:, j, :], start=st, stop=sp)
                nc.tensor.matmul(Vp[b][:, :], vcs[b][:, j, :], wv_c[:, j, :], start=st, stop=sp)

    # prefetch fc weights early (no data deps; sync queue is idle after phase A)
    wfcts = []
    for pt in range(NPC):
        wfct = fcpool.tile([128, 4, 512], act_dt, tag="wfct", name="wfct")
        nc.sync.dma_start(out=wfct, in_=io["wfc"][pt])
        wfcts.append(wfct)

    # ---- evacuate PSUM: copies for both batches free all 6 proj banks ----
    qkv_sb = []
    for b in range(BPC):
        Q_sb = sb.tile([128, D], act_dt, tag="Q_sb", name="Q_sb")
        K_sb = sb.tile([128, D], act_dt, tag="K_sb", name="K_sb")
        V_sb = sb.tile([128, D], act_dt, tag="V_sb", name="V_sb")
        nc.vector.tensor_copy(out=Q_sb, in_=Qp[b][:, :])
        nc.scalar.copy(out=K_sb, in_=Kp[b][:, :])
        nc.vector.tensor_copy(out=V_sb, in_=Vp[b][:, :])
        qkv_sb.append((Q_sb, K_sb, V_sb))
    ctx_a.__exit__(None, None, None)
    ps_s = ctx.enter_context(tc.tile_pool(name="ps_s", bufs=2, space="PSUM"))
    ps_o = ctx.enter_context(tc.tile_pool(name="ps_o", bufs=2, space="PSUM"))
    ps_fc = ctx.enter_context(tc.tile_pool(name="ps_fc", bufs=2, space="PSUM"))

    # per-channel partial sums: cols 0..15 = sum(out) per (b,pt), 16..31 = sum(out^2)
    pcols = stat.tile([128, 32], F32, tag="pcols", name="pcols")

    # ---- phases B-D per batch: attention, silu+LN ----
    xTs = []
    for b in range(BPC):
        Q_sb, K_sb, V_sb = qkv_sb[b]

        QT_sb = sb.tile([128, D], act_dt, tag="QT_sb", name="QT_sb")
        KT_sb = sb.tile([128, D], act_dt, tag="KT_sb", name="KT_sb")
        for src, dst in ((Q_sb, QT_sb), (K_sb, KT_sb)):
            for dc in range(4):
                tp = ps_s.tile([128, 128], act_dt, tag="stp", name="stp")
                nc.tensor.transpose(tp[:, :], src[:, dc * 128:(dc + 1) * 128], ident[:, :])
                nc.vector.tensor_copy(out=dst[:, dc * 128:(dc + 1) * 128], in_=raw(tp[:, :]))

        Opsum = ps_o.tile([128, D], F32, tag="O", name="O")
        Osc = sb.tile([128, D], F32, tag="Osc", name="Osc")
        for h in range(NH):
            po = (h % 2) * 64
            fo = (h // 2) * 128
            S = ps_s.tile([128, 128], F32, tag="S", name="S")
            nc.tensor.matmul(S[:, :], QT_sb[po:po + 64, fo:fo + 128],
                             KT_sb[po:po + 64, fo:fo + 128], start=True, stop=True)
            e_f = sb.tile([128, 128], F32, tag="e_f", name="e_f")
            lsum = small.tile([128, 1], F32, tag="lsum", name="lsum")
            nc.scalar.activation(out=e_f, in_=S[:, :], func=AF.Exp, accum_out=lsum)
            rs = small.tile([128, 1], F32, tag="rs", name="rs")
            nc.vector.reciprocal(rs, lsum)
            tpa = ps_s.tile([128, 128], F32, tag="stp", name="stp")
            nc.tensor.transpose(tpa[:, :], e_f[:, :], ident_f[:, :])
            aT = sb.tile([128, 128], act_dt, tag="aT", name="aT")
            nc.scalar.copy(out=aT, in_=tpa[:, :])
            nc.tensor.matmul(Opsum[:, h * 64:(h + 1) * 64], aT[:, :],
                             V_sb[:, h * 64:(h + 1) * 64], start=True, stop=True)
            nc.vector.tensor_scalar_mul(out=Osc[:, h * 64:(h + 1) * 64],
                                        in0=Opsum[:, h * 64:(h + 1) * 64],
                                        scalar1=rs)

        # silu + layernorm (affine folded into fc weights on host)
        sg = sb.tile([128, D], F32, tag="sg", name="sg")
        nc.scalar.activation(out=sg, in_=Osc, func=AF.Sigmoid)
        Osw = sb.tile([128, D], F32, tag="Osw", name="Osw")
        nc.vector.tensor_mul(out=Osw, in0=Osc, in1=sg)
        st6 = small.tile([128, 6], F32, tag="st6", name="st6")
        nc.vector.bn_stats(out=st6, in_=Osw)
        mv = small.tile([128, 2], F32, tag="mv", name="mv")
        nc.vector.bn_aggr(out=mv, in_=st6)
        sd = small.tile([128, 1], F32, tag="sd", name="sd")
        nc.scalar.activation(out=sd, in_=mv[:, 1:2], func=AF.Sqrt, scale=float(D) / (D - 1))
        nc.vector.tensor_scalar_add(out=sd, in0=sd, scalar1=LN_EPS)
        rstd = small.tile([128, 1], F32, tag="rstd", name="rstd")
        nc.vector.reciprocal(rstd, sd)
        xhat = sb.tile([128, D], act_dt, tag="xhat", name="xhat")
        nc.vector.tensor_scalar(out=xhat, in0=Osw, scalar1=mv[:, 0:1], scalar2=rstd,
                                op0=ALU.subtract, op1=ALU.mult)
        xT = sb.tile([128, D], act_dt, tag="xT", name="xT")
        for dc in range(4):
            tp = ps_s.tile([128, 128], act_dt, tag="stp", name="stp")
            nc.tensor.transpose(tp[:, :], xhat[:, dc * 128:(dc + 1) * 128], ident[:, :])
            nc.vector.tensor_copy(out=xT[:, dc * 128:(dc + 1) * 128], in_=raw(tp[:, :]))
        xTs.append(xT)

    # ---- phase D2: fc + residual + BN partial sums, streaming wfc ----
    for pt in range(NPC):
        for b in range(BPC):
            O2 = ps_fc.tile([128, 512], F32, tag="O2", name="O2")
            for dc in range(4):
                nc.tensor.matmul(O2[:, :], xTs[b][:, dc * 128:(dc + 1) * 128],
                                 wfcts[pt][:, dc, :], start=dc == 0, stop=dc == 3)
            seg = out_sb[b][:, pt * 512:(pt + 1) * 512]
            nc.vector.tensor_add(out=seg, in0=seg, in1=O2[:, :])
            nc.vector.reduce_sum(pcols[:, b * NPC + pt:b * NPC + pt + 1], seg, axis=AX.X)
            junk = sb.tile([128, 512], F32, tag="junk", name="junk")
            nc.scalar.activation(out=junk, in_=seg, func=AF.Square,
                                 accum_out=pcols[:, 16 + b * NPC + pt:17 + b * NPC + pt])

    # ---- phase E: BN stats AllReduce + normalize + store ----
    stats2 = stat.tile([128, 2], F32, tag="stats2", name="stats2")
    nc.vector.reduce_sum(stats2[:, 0:1], pcols[:, 0:16], axis=AX.X)
    nc.vector.reduce_sum(stats2[:, 1:2], pcols[:, 16:32], axis=AX.X)

    cin = dram.tile([128, 2], F32, tag="cin", name="cin")
    cout = dram.tile([128, 2], F32, tag="cout", name="cout")
    nc.gpsimd.dma_start(out=cin[:, :], in_=stats2)
    if os.environ.get("BASS_SKIP_COLL", "0") == "1":
        nc.gpsimd.dma_start(out=cout[:, :], in_=cin[:, :])
    else:
        nc.gpsimd.collective_compute(
            "AllReduce",
            ALU.add,
            replica_groups=[list(range(N_CORES))],
            ins=[cin.opt()],
            outs=[cout.opt()],
        )
    red = stat.tile([128, 2], F32, tag="red", name="red")
    nc.gpsimd.dma_start(out=red[:, :], in_=cout[:, :])

    inv_n = 1.0 / float(B * P)
    mean = small.tile([128, 1], F32, tag="mean", name="mean")
    nc.scalar.mul(out=mean, in_=red[:, 0:1], mul=inv_n)
    ex2 = small.tile([128, 1], F32, tag="ex2", name="ex2")
    nc.scalar.mul(out=ex2, in_=red[:, 1:2], mul=inv_n)
    msq = small.tile([128, 1], F32, tag="msq", name="msq")
    nc.vector.tensor_mul(out=msq, in0=mean, in1=mean)
    var = small.tile([128, 1], F32, tag="var", name="var")
    nc.vector.tensor_sub(out=var, in0=ex2, in1=msq)
    epsbn = consts.tile([128, 1], F32, tag="epsbn", name="epsbn")
    nc.vector.memset(epsbn, BN_EPS)
    sdv = small.tile([128, 1], F32, tag="sdv", name="sdv")
    nc.scalar.activation(out=sdv, in_=var, func=AF.Sqrt, bias=epsbn)
    invs = small.tile([128, 1], F32, tag="invs", name="invs")
    nc.vector.reciprocal(invs, sdv)
    scl = small.tile([128, 1], F32, tag="scl", name="scl")
    nc.vector.tensor_mul(out=scl, in0=bng, in1=invs)
    tmp = small.tile([128, 1], F32, tag="tmp", name="tmp")
    nc.vector.tensor_mul(out=tmp, in0=mean, in1=scl)
    shf = small.tile([128, 1], F32, tag="shf", name="shf")
    nc.vector.tensor_sub(out=shf, in0=bnb, in1=tmp)

    for b in range(BPC):
        for pt in range(NPC):
            seg = out_sb[b][:, pt * 512:(pt + 1) * 512]
            nc.vector.tensor_scalar(out=seg, in0=seg, scalar1=scl, scalar2=shf,
                                    op0=ALU.mult, op1=ALU.add)
            nc.gpsimd.dma_start(out=io["out"][b, :, pt * 512:(pt + 1) * 512], in_=seg)


def _build():
    key = (MODE, os.environ.get("BASS_SKIP_COLL", "0"))
    if key in _BUILD_CACHE:
        return _BUILD_CACHE[key]
    act_np = {"f32r": F32R, "bf16": BF16, "f32": F32}[MODE]
    nc = bacc.Bacc("TRN2", target_bir_lowering=False, debug=False, num_devices=N_CORES)
    io = {
        "qT": nc.dram_tensor("qT", [BPC, NPC, 128, 4, 128], act_np, kind="ExternalInput").ap(),
        "kT": nc.dram_tensor("kT", [BPC, NPC, 128, 4, 128], act_np, kind="ExternalInput").ap(),
        "vT": nc.dram_tensor("vT", [BPC, NPC, 128, 4, 128], act_np, kind="ExternalInput").ap(),
        "veff": nc.dram_tensor("veff", [BPC, C, P], F32, kind="ExternalInput").ap(),
        "wq": nc.dram_tensor("wq", [NPC, 128, 4, D], act_np, kind="ExternalInput").ap(),
        "wk": nc.dram_tensor("wk", [NPC, 128, 4, D], act_np, kind="ExternalInput").ap(),
        "wv": nc.dram_tensor("wv", [NPC, 128, 4, D], act_np, kind="ExternalInput").ap(),
        "wfc": nc.dram_tensor("wfc", [NPC, 128, 4, 512], act_np, kind="ExternalInput").ap(),
        "bng": nc.dram_tensor("bng", [C, 1], F32, kind="ExternalInput").ap(),
        "bnb": nc.dram_tensor("bnb", [C, 1], F32, kind="ExternalInput").ap(),
        "out": nc.dram_tensor("out", [BPC, C, P], F32, kind="ExternalOutput").ap(),
    }
    from contextlib import ExitStack
    with tile.TileContext(nc) as tc, ExitStack() as ctx:
        _emit(ctx, nc, tc, io)
    nc.compile()
    _BUILD_CACHE[key] = nc
    return nc


def _np_cast(x):
    if MODE == "bf16":
        import ml_dtypes
        return np.ascontiguousarray(np.asarray(x, np.float32).astype(ml_dtypes.bfloat16))
    return np.ascontiguousarray(np.asarray(x, np.float32))


def _pack_acts(xT):
    # [b, 4096, 128] -> [b, NPC, 128, 4, 128]  (pc-chunk, partition, j, c)
    b = xT.shape[0]
    return np.ascontiguousarray(
        xT.reshape(b, NPC, 4, 128, 128).transpose(0, 1, 3, 2, 4))


def _pack_w(w):
    # [4096, D] -> [NPC, 128, 4, D]
    return np.ascontiguousarray(w.reshape(NPC, 4, 128, -1).transpose(0, 2, 1, 3))


def kernel(v, k, q, w_qs, w_ks, w_vs, w_fc, ln_gamma, ln_beta, temperature,
           bn_gamma, bn_beta, **_ignored):
    v = np.asarray(v, np.float32)
    k = np.asarray(k, np.float32)
    q = np.asarray(q, np.float32)
    w_qs = np.asarray(w_qs, np.float32)
    w_ks = np.asarray(w_ks, np.float32)
    w_vs = np.asarray(w_vs, np.float32)
    w_fc = np.asarray(w_fc, np.float32)
    ln_gamma = np.asarray(ln_gamma, np.float32)
    ln_beta = np.asarray(ln_beta, np.float32)
    temp = float(np.asarray(temperature))
    bn_gamma = np.asarray(bn_gamma, np.float32)
    bn_beta = np.asarray(bn_beta, np.float32)

    qf = q.reshape(B, C, P)
    kf = k.reshape(B, C, P)
    vf = v.reshape(B, C, P)
    qT = _np_cast(_pack_acts(qf.transpose(0, 2, 1)))
    kT = _np_cast(_pack_acts(kf.transpose(0, 2, 1)))
    vT = _np_cast(_pack_acts(vf.transpose(0, 2, 1)))
    wq = _np_cast(_pack_w((w_qs / temp).T))
    wk = _np_cast(_pack_w(w_ks.T))
    wv = _np_cast(_pack_w(w_vs.T))
    # wfc packed as [pt, p, dc, c]: wfcT_eff[dc*128+p, pt*512+c]
    wfcT_eff = (w_fc * ln_gamma[None, :]).T  # [D, P]
    wfc = _np_cast(wfcT_eff.reshape(4, 128, NPC, 512).transpose(2, 1, 0, 3))
    bias_fc = (w_fc @ ln_beta).astype(np.float32)
    veff = np.ascontiguousarray(vf + bias_fc[None, None, :])
    bng = np.ascontiguousarray(bn_gamma.reshape(C, 1))
    bnb = np.ascontiguousarray(bn_beta.reshape(C, 1))

    nc = _build()
    in_maps = []
    for i in range(N_CORES):
        bs = slice(BPC * i, BPC * (i + 1))
        in_maps.append({
            "qT": qT[bs], "kT": kT[bs], "vT": vT[bs], "veff": veff[bs],
            "wq": wq, "wk": wk, "wv": wv, "wfc": wfc,
            "bng": bng, "bnb": bnb,
        })
    res = run_bass_kernel_spmd(nc, in_maps, core_ids=list(range(N_CORES)))
    global LAST_RESULTS
    LAST_RESULTS = res
    out = np.concatenate([res.results[i]["out"] for i in range(N_CORES)], axis=0)
    return out.reshape(B, C, HH, WW).astype(np.float32)



# revision 19
# speedup vs baseline: 1.0165x; 1.0165x over previous
"""Trainium2 Bass kernel for nn_MultiHeadAttention (channel-attention transformer block).

Math (per batch b, with X* = reshape(*, [C, P]), P = 4096, C = 128, D = 512):
  Q = Xq @ (Wq/temp)^T, K = Xk @ Wk^T, V = Xv @ Wv^T            [C, D]
  per head h (8 heads, ld=64): A_h = softmax(Q_h K_h^T); O_h = A_h V_h
  O = silu(O); O = (O - mean)/(unbiased_std + eps)   (LN affine folded into fc)
  out_pre = (v + Wfc@ln_beta) + O @ (Wfc*ln_gamma)^T
  out = BatchNorm2d(out_pre)   (batch stats over (b,h,w), biased var)

Sharding: data-parallel over batch, 2 batches per core on 8 cores; BatchNorm
statistics combined with a tiny AllReduce ([128,2] per core).  A warm-up
AllReduce at kernel start absorbs cross-core launch skew + ncfw wakeup so the
real AllReduce at the end runs near its latency floor.

v2 notes:
 - bf16 everywhere (weights, activations, residual, output store).
 - residual added into the fc PSUM via an identity-matmul accumulation step.
 - BN per-channel sums come from host-precomputed veff sums + a tiny
   xhatT @ colsum(wfc) matmul; only sum-of-squares needs a Scalar pass.
 - fc weights + residual stream in during the attention phase (DMA idle there).
"""

import os

import numpy as np

import concourse.mybir as mybir
import concourse.tile as tile
from concourse import bacc
from concourse.bass_utils import run_bass_kernel_spmd
from concourse.masks import make_identity

# ---- problem constants (hardcoded per contract) ----
B, C, HH, WW = 16, 128, 64, 64
P = HH * WW           # 4096
NH, LD = 8, 64
D = NH * LD           # 512
N_CORES = 8
BPC = B // N_CORES    # 2 batches per core
NPC = P // 512        # 8 quad-chunks over contraction / output tiles
LN_EPS = 1e-6
BN_EPS = 1e-5
F32 = mybir.dt.float32
BF16 = mybir.dt.bfloat16

_BUILD_CACHE: dict = {}
LAST_RESULTS = None  # BassKernelResults of the most recent run (for profiling)

SKIP_COLL = os.environ.get("BASS_SKIP_COLL", "0") == "1"
WARM_AR = os.environ.get("BASS_WARM_AR", "1") == "1"
MODE = "bf16"  # kept for test harness printing


def _emit(ctx, nc, tc, io):
    AF = mybir.ActivationFunctionType
    ALU = mybir.AluOpType
    AX = mybir.AxisListType

    consts = ctx.enter_context(tc.tile_pool(name="consts", bufs=1))
    wpool = ctx.enter_context(tc.tile_pool(name="wpool", bufs=2))
    fcpool = ctx.enter_context(tc.tile_pool(name="fcpool", bufs=1))
    apool = ctx.enter_context(tc.tile_pool(name="apool", bufs=2))
    sb = ctx.enter_context(tc.tile_pool(name="sb", bufs=2))
    small = ctx.enter_context(tc.tile_pool(name="small", bufs=4))
    stat = ctx.enter_context(tc.tile_pool(name="stat", bufs=1))
    dram = ctx.enter_context(tc.tile_pool(name="dram", bufs=1, space="DRAM"))

    # identity for PE transposes / residual accumulation
    ident_f = consts.tile([128, 128], F32, tag="identf", name="identf")
    make_identity(nc, ident_f)
    ident = consts.tile([128, 128], BF16, tag="ident", name="ident")
    nc.vector.tensor_copy(out=ident, in_=ident_f)

    # ---- warm-up AllReduce: absorbs launch skew + wakes ncfw early ----
    if WARM_AR and not SKIP_COLL:
        wsb = consts.tile([128, 2], F32, tag="wsb", name="wsb")
        nc.vector.memset(wsb, 0.0)
        cwin = dram.tile([128, 2], F32, tag="cwin", name="cwin")
        cwout = dram.tile([128, 2], F32, tag="cwout", name="cwout")
        nc.gpsimd.dma_start(out=cwin[:, :], in_=wsb)
        nc.gpsimd.collective_compute(
            "AllReduce",
            ALU.add,
            replica_groups=[list(range(N_CORES))],
            ins=[cwin.opt()],
            outs=[cwout.opt()],
        )

    # ---- phase A: QKV projections, accumulating over the P=4096 contraction ----
    ps_proj_ctx = tc.tile_pool(name="ps_proj", bufs=1, space="PSUM")
    ps_proj = ps_proj_ctx.__enter__()
    warm = ps_proj.tile([128, 128], BF16, tag="warm", name="warm")
    nc.tensor.transpose(warm[:, :], ident[:, :], ident[:, :])
    Qp = [ps_proj.tile([128, D], F32, tag=f"Qp{b}", name=f"Qp{b}") for b in range(BPC)]
    Kp = [ps_proj.tile([128, D], F32, tag=f"Kp{b}", name=f"Kp{b}") for b in range(BPC)]
    Vp = [ps_proj.tile([128, D], F32, tag=f"Vp{b}", name=f"Vp{b}") for b in range(BPC)]

    for pc in range(NPC):
        wq_c = wpool.tile([128, 4, D], BF16, tag="wq_c", name="wq_c")
        wk_c = wpool.tile([128, 4, D], BF16, tag="wk_c", name="wk_c")
        wv_c = wpool.tile([128, 4, D], BF16, tag="wv_c", name="wv_c")
        nc.sync.dma_start(out=wq_c, in_=io["wq"][pc])
        nc.scalar.dma_start(out=wk_c, in_=io["wk"][pc])
        nc.gpsimd.dma_start(out=wv_c, in_=io["wv"][pc])
        qcs, kcs, vcs = [], [], []
        for b in range(BPC):
            qc = apool.tile([128, 4, 128], BF16, tag=f"qc{b}", name=f"qc{b}")
            kc = apool.tile([128, 4, 128], BF16, tag=f"kc{b}", name=f"kc{b}")
            vc = apool.tile([128, 4, 128], BF16, tag=f"vc{b}", name=f"vc{b}")
            nc.sync.dma_start(out=qc, in_=io["qT"][b, pc])
            nc.scalar.dma_start(out=kc, in_=io["kT"][b, pc])
            nc.gpsimd.dma_start(out=vc, in_=io["vT"][b, pc])
            qcs.append(qc); kcs.append(kc); vcs.append(vc)
        for j in range(4):
            st = pc == 0 and j == 0
            sp = pc == NPC - 1 and j == 3
            for b in range(BPC):
                nc.tensor.matmul(Qp[b][:, :], qcs[b][:, j, :], wq_c[:, j, :], start=st, stop=sp)
                nc.tensor.matmul(Kp[b][:, :], kcs[b][:, j, :], wk_c[:, j, :], start=st, stop=sp)
                nc.tensor.matmul(Vp[b][:, :], vcs[b][:, j, :], wv_c[:, j, :], start=st, stop=sp)

    # prefetch fc weights, residual, colsums during the attention phase
    # (queued behind phase-A transfers; DMA is otherwise idle there)
    wfcts = []
    for pt in range(NPC):
        wfct = fcpool.tile([128, 4, 512], BF16, tag=f"wfct{pt}", name=f"wfct{pt}")
        nc.sync.dma_start(out=wfct, in_=io["wfc"][pt])
        wfcts.append(wfct)
    resid = []
    for b in range(BPC):
        t = fcpool.tile([128, NPC, 512], BF16, tag=f"resid{b}", name=f"resid{b}")
        nc.scalar.dma_start(out=t, in_=io["resid"][b])
        resid.append(t)
    colsum = consts.tile([128, 4, NPC], BF16, tag="colsum", name="colsum")
    nc.gpsimd.dma_start(out=colsum, in_=io["colsum"][:, :, :])
    vsums = consts.tile([128, 2 * NPC], F32, tag="vsums", name="vsums")
    nc.gpsimd.dma_start(out=vsums, in_=io["vsums"][:, :])
    bng = consts.tile([128, 1], F32, tag="bng", name="bng")
    bnb = consts.tile([128, 1], F32, tag="bnb", name="bnb")
    nc.gpsimd.dma_start(out=bng, in_=io["bng"][:, :])
    nc.gpsimd.dma_start(out=bnb, in_=io["bnb"][:, :])

    # ---- evacuate PSUM: Q/K/V copies for both batches free all 6 proj banks ----
    qkv_sb = []
    for b in range(BPC):
        Q_sb = sb.tile([128, D], BF16, tag="Q_sb", name="Q_sb")
        K_sb = sb.tile([128, D], BF16, tag="K_sb", name="K_sb")
        V_sb = sb.tile([128, D], BF16, tag="V_sb", name="V_sb")
        nc.vector.tensor_copy(out=Q_sb, in_=Qp[b][:, :])
        nc.scalar.copy(out=K_sb, in_=Kp[b][:, :])
        nc.vector.tensor_copy(out=V_sb, in_=Vp[b][:, :])
        qkv_sb.append((Q_sb, K_sb, V_sb))
    ps_proj_ctx.__exit__(None, None, None)
    # attention-era PSUM: 3 rotating scratch banks (S scores / transposes) +
    # 2 Opsum banks; closed before the fc-era pools open (8-bank budget)
    ps_s_ctx = tc.tile_pool(name="ps_s", bufs=3, space="PSUM")
    ps_s = ps_s_ctx.__enter__()
    ps_o_ctx = tc.tile_pool(name="ps_o", bufs=1, space="PSUM")
    ps_o = ps_o_ctx.__enter__()

    # ---- phase B: Q/K transposes + attention, head-interleaved across batches ----
    QTs, KTs = [], []
    for b in range(BPC):
        Q_sb, K_sb, V_sb = qkv_sb[b]
        QT_sb = sb.tile([128, D], BF16, tag="QT_sb", name="QT_sb")
        KT_sb = sb.tile([128, D], BF16, tag="KT_sb", name="KT_sb")
        for si, (src, dst) in enumerate(((Q_sb, QT_sb), (K_sb, KT_sb))):
            for dc in range(4):
                tp = ps_s.tile([128, 128], BF16, tag="sm", name="stp")
                nc.tensor.transpose(tp[:, :], src[:, dc * 128:(dc + 1) * 128], ident[:, :])
                eng = nc.vector if (si + dc) % 2 == 0 else nc.scalar
                if eng is nc.vector:
                    nc.vector.tensor_copy(out=dst[:, dc * 128:(dc + 1) * 128], in_=tp[:, :])
                else:
                    nc.scalar.copy(out=dst[:, dc * 128:(dc + 1) * 128], in_=tp[:, :])
        QTs.append(QT_sb); KTs.append(KT_sb)

    Opsums = [ps_o.tile([128, D], F32, tag=f"O{b}", name=f"O{b}") for b in range(BPC)]
    Oscs = [sb.tile([128, D], F32, tag=f"Osc{b}", name=f"Osc{b}") for b in range(BPC)]
    for h in range(NH):
        po = (h % 2) * 64
        fo = (h // 2) * 128
        for b in range(BPC):
            S = ps_s.tile([128, 128], F32, tag="sm", name="S")
            nc.tensor.matmul(S[:, :], QTs[b][po:po + 64, fo:fo + 128],
                             KTs[b][po:po + 64, fo:fo + 128], start=True, stop=True)
            e_f = sb.tile([128, 128], BF16, tag="e_f", name="e_f")
            lsum = small.tile([128, 1], F32, tag="lsum", name="lsum")
            nc.scalar.activation(out=e_f, in_=S[:, :], func=AF.Exp, accum_out=lsum)
            rs = small.tile([128, 1], F32, tag="rs", name="rs")
            nc.vector.reciprocal(rs, lsum)
            tpa = ps_s.tile([128, 128], BF16, tag="sm", name="stp")
            nc.tensor.transpose(tpa[:, :], e_f[:, :], ident[:, :])
            aT = sb.tile([128, 128], BF16, tag="aT", name="aT")
            if b == 0:
                nc.vector.tensor_copy(out=aT, in_=tpa[:, :])
            else:
                nc.scalar.copy(out=aT, in_=tpa[:, :])
            nc.tensor.matmul(Opsums[b][:, h * 64:(h + 1) * 64], aT[:, :],
                             qkv_sb[b][2][:, h * 64:(h + 1) * 64], start=True, stop=True)
            # softmax denominator: O / sum  (Vector; Silu batched later to
            # avoid per-head ACT table-set thrash between Exp and Silu)
            nc.vector.tensor_scalar_mul(out=Oscs[b][:, h * 64:(h + 1) * 64],
                                        in0=Opsums[b][:, h * 64:(h + 1) * 64],
                                        scalar1=rs)

    # ---- phase C: silu + LayerNorm (affine folded into fc weights on host) ----
    # batched by ACT table-set: all Silus, then all Sqrts
    Osws, mvs = [], []
    for b in range(BPC):
        Osw = sb.tile([128, D], F32, tag=f"Osw{b}", name=f"Osw{b}")
        nc.scalar.activation(out=Osw, in_=Oscs[b], func=AF.Silu)
        st6 = small.tile([128, 6], F32, tag="st6", name="st6")
        nc.vector.bn_stats(out=st6, in_=Osw)
        mv = small.tile([128, 2], F32, tag=f"mv{b}", name=f"mv{b}")
        nc.vector.bn_aggr(out=mv, in_=st6)
        Osws.append(Osw); mvs.append(mv)
    xTs = []
    for b in range(BPC):
        Osw, mv = Osws[b], mvs[b]
        sd = small.tile([128, 1], F32, tag="sd", name="sd")
        nc.scalar.activation(out=sd, in_=mv[:, 1:2], func=AF.Sqrt, scale=float(D) / (D - 1))
        nc.vector.tensor_scalar_add(out=sd, in0=sd, scalar1=LN_EPS)
        rstd = small.tile([128, 1], F32, tag="rstd", name="rstd")
        nc.vector.reciprocal(rstd, sd)
        xhat = sb.tile([128, D], BF16, tag="xhat", name="xhat")
        nc.vector.tensor_scalar(out=xhat, in0=Osw, scalar1=mv[:, 0:1], scalar2=rstd,
                                op0=ALU.subtract, op1=ALU.mult)
        xT = sb.tile([128, D], BF16, tag="xT", name="xT")
        for dc in range(4):
            tp = ps_s.tile([128, 128], BF16, tag="sm", name="stp")
            nc.tensor.transpose(tp[:, :], xhat[:, dc * 128:(dc + 1) * 128], ident[:, :])
            if dc % 2 == 0:
                nc.vector.tensor_copy(out=xT[:, dc * 128:(dc + 1) * 128], in_=tp[:, :])
            else:
                nc.scalar.copy(out=xT[:, dc * 128:(dc + 1) * 128], in_=tp[:, :])
        xTs.append(xT)

    # attention-era PSUM closes; fc-era pools open (O2 x2 + fcS x2 = 4 banks)
    ps_o_ctx.__exit__(None, None, None)
    ps_s_ctx.__exit__(None, None, None)
    ps_fc = ctx.enter_context(tc.tile_pool(name="ps_fc", bufs=2, space="PSUM"))
    ps_fs = ctx.enter_context(tc.tile_pool(name="ps_fs", bufs=1, space="PSUM"))

    # per-channel fc-output sums via colsum matmul: fcS[b][c, pt] = sum_d xhat*colsum
    fcS = []
    for b in range(BPC):
        f = ps_fs.tile([128, NPC], F32, tag=f"fcS{b}", name=f"fcS{b}")
        for dc in range(4):
            nc.tensor.matmul(f[:, :], xTs[b][:, dc * 128:(dc + 1) * 128],
                             colsum[:, dc, :], start=dc == 0, stop=dc == 3)
        fcS.append(f)

    # ---- phase D: fc + residual(identity-matmul) + sumsq accumulation ----
    # seg tiles (bf16) hold pre-BN outputs until the AllReduce lands
    segs = [sb.tile([128, NPC, 512], BF16, tag=f"seg{b}", name=f"seg{b}")
            for b in range(BPC)]
    pcols = stat.tile([128, 16], F32, tag="pcols", name="pcols")
    for pt in range(NPC):
        for b in range(BPC):
            O2 = ps_fc.tile([128, 512], F32, tag="O2", name="O2")
            for dc in range(4):
                nc.tensor.matmul(O2[:, :], xTs[b][:, dc * 128:(dc + 1) * 128],
                                 wfcts[pt][:, dc, :], start=dc == 0, stop=False)
            nc.tensor.matmul(O2[:, :], ident[:, :], resid[b][:, pt, :],
                             start=False, stop=True)
            seg = segs[b][:, pt, :]
            if b == 0:
                nc.vector.tensor_copy(out=seg, in_=O2[:, :])
            else:
                nc.scalar.copy(out=seg, in_=O2[:, :])
            junk = sb.tile([128, 512], BF16, tag="junk", name="junk")
            nc.scalar.activation(out=junk, in_=O2[:, :], func=AF.Square,
                                 accum_out=pcols[:, b * NPC + pt:b * NPC + pt + 1])

    # ---- phase E: BN stats AllReduce + normalize + store ----
    stats2 = stat.tile([128, 2], F32, tag="stats2", name="stats2")
    ssum = stat.tile([128, NPC], F32, tag="ssum", name="ssum")
    nc.vector.tensor_add(out=ssum, in0=vsums[:, 0:NPC], in1=vsums[:, NPC:2 * NPC])
    nc.vector.tensor_add(out=ssum, in0=ssum, in1=fcS[0][:, :])
    nc.vector.tensor_add(out=ssum, in0=ssum, in1=fcS[1][:, :])
    nc.vector.reduce_sum(stats2[:, 0:1], ssum, axis=AX.X)
    nc.vector.reduce_sum(stats2[:, 1:2], pcols[:, 0:16], axis=AX.X)

    cin = dram.tile([128, 2], F32, tag="cin", name="cin")
    cout = dram.tile([128, 2], F32, tag="cout", name="cout")
    nc.gpsimd.dma_start(out=cin[:, :], in_=stats2)
    if SKIP_COLL:
        nc.gpsimd.dma_start(out=cout[:, :], in_=cin[:, :])
    else:
        nc.gpsimd.collective_compute(
            "AllReduce",
            ALU.add,
            replica_groups=[list(range(N_CORES))],
            ins=[cin.opt()],
            outs=[cout.opt()],
        )
    red = stat.tile([128, 2], F32, tag="red", name="red")
    nc.gpsimd.dma_start(out=red[:, :], in_=cout[:, :])

    inv_n = 1.0 / float(B * P)
    epsbn = consts.tile([128, 1], F32, tag="epsbn", name="epsbn")
    nc.vector.memset(epsbn, BN_EPS)
    me = stat.tile([128, 2], F32, tag="me", name="me")  # [mean, E[x^2]]
    nc.vector.tensor_scalar_mul(out=me, in0=red[:, :], scalar1=inv_n)
    msq = small.tile([128, 1], F32, tag="msq", name="msq")
    nc.vector.tensor_mul(out=msq, in0=me[:, 0:1], in1=me[:, 0:1])
    var = small.tile([128, 1], F32, tag="var", name="var")
    nc.vector.tensor_sub(out=var, in0=me[:, 1:2], in1=msq)
    sdv = small.tile([128, 1], F32, tag="sdv", name="sdv")
    nc.scalar.activation(out=sdv, in_=var, func=AF.Sqrt, bias=epsbn)
    invs = small.tile([128, 1], F32, tag="invs", name="invs")
    nc.vector.reciprocal(invs, sdv)
    scl = small.tile([128, 1], F32, tag="scl", name="scl")
    nc.vector.tensor_mul(out=scl, in0=bng, in1=invs)
    tmp = small.tile([128, 1], F32, tag="tmp", name="tmp")
    nc.vector.tensor_mul(out=tmp, in0=me[:, 0:1], in1=scl)
    shf = small.tile([128, 1], F32, tag="shf", name="shf")
    nc.vector.tensor_sub(out=shf, in0=bnb, in1=tmp)

    dq = [nc.gpsimd, nc.sync, nc.scalar]
    for pt in range(NPC):
        for b in range(BPC):
            seg = segs[b][:, pt, :]
            if (pt + b) % 2 == 0:
                nc.vector.tensor_scalar(out=seg, in0=seg, scalar1=scl, scalar2=shf,
                                        op0=ALU.mult, op1=ALU.add)
            else:
                nc.scalar.activation(out=seg, in_=seg, func=AF.Identity,
                                     scale=scl, bias=shf)
            dq[(pt * BPC + b) % 3].dma_start(out=io["out"][b, :, pt * 512:(pt + 1) * 512],
                                             in_=seg)


def _build():
    key = (SKIP_COLL, WARM_AR)
    if key in _BUILD_CACHE:
        return _BUILD_CACHE[key]
    nc = bacc.Bacc("TRN2", target_bir_lowering=False, debug=False, num_devices=N_CORES)
    io = {
        "qT": nc.dram_tensor("qT", [BPC, NPC, 128, 4, 128], BF16, kind="ExternalInput").ap(),
        "kT": nc.dram_tensor("kT", [BPC, NPC, 128, 4, 128], BF16, kind="ExternalInput").ap(),
        "vT": nc.dram_tensor("vT", [BPC, NPC, 128, 4, 128], BF16, kind="ExternalInput").ap(),
        "resid": nc.dram_tensor("resid", [BPC, C, NPC, 512], BF16, kind="ExternalInput").ap(),
        "wq": nc.dram_tensor("wq", [NPC, 128, 4, D], BF16, kind="ExternalInput").ap(),
        "wk": nc.dram_tensor("wk", [NPC, 128, 4, D], BF16, kind="ExternalInput").ap(),
        "wv": nc.dram_tensor("wv", [NPC, 128, 4, D], BF16, kind="ExternalInput").ap(),
        "wfc": nc.dram_tensor("wfc", [NPC, 128, 4, 512], BF16, kind="ExternalInput").ap(),
        "colsum": nc.dram_tensor("colsum", [128, 4, NPC], BF16, kind="ExternalInput").ap(),
        "vsums": nc.dram_tensor("vsums", [C, 2 * NPC], F32, kind="ExternalInput").ap(),
        "bng": nc.dram_tensor("bng", [C, 1], F32, kind="ExternalInput").ap(),
        "bnb": nc.dram_tensor("bnb", [C, 1], F32, kind="ExternalInput").ap(),
        "out": nc.dram_tensor("out", [BPC, C, P], BF16, kind="ExternalOutput").ap(),
    }
    from contextlib import ExitStack
    with tile.TileContext(nc) as tc, ExitStack() as ctx:
        _emit(ctx, nc, tc, io)
    nc.compile()
    _BUILD_CACHE[key] = nc
    return nc


def _bf16(x):
    import ml_dtypes
    return np.ascontiguousarray(np.asarray(x, np.float32).astype(ml_dtypes.bfloat16))


def _pack_acts(xT):
    # [b, 4096, 128] -> [b, NPC, 128, 4, 128]  (pc-chunk, partition, j, c)
    b = xT.shape[0]
    return np.ascontiguousarray(
        xT.reshape(b, NPC, 4, 128, 128).transpose(0, 1, 3, 2, 4))


def _pack_w(w):
    # [4096, D] -> [NPC, 128, 4, D]
    return np.ascontiguousarray(w.reshape(NPC, 4, 128, -1).transpose(0, 2, 1, 3))


def kernel(v, k, q, w_qs, w_ks, w_vs, w_fc, ln_gamma, ln_beta, temperature,
           bn_gamma, bn_beta, **_ignored):
    v = np.asarray(v, np.float32)
    k = np.asarray(k, np.float32)
    q = np.asarray(q, np.float32)
    w_qs = np.asarray(w_qs, np.float32)
    w_ks = np.asarray(w_ks, np.float32)
    w_vs = np.asarray(w_vs, np.float32)
    w_fc = np.asarray(w_fc, np.float32)
    ln_gamma = np.asarray(ln_gamma, np.float32)
    ln_beta = np.asarray(ln_beta, np.float32)
    temp = float(np.asarray(temperature))
    bn_gamma = np.asarray(bn_gamma, np.float32)
    bn_beta = np.asarray(bn_beta, np.float32)

    qf = q.reshape(B, C, P)
    kf = k.reshape(B, C, P)
    vf = v.reshape(B, C, P)
    qT = _bf16(_pack_acts(qf.transpose(0, 2, 1)))
    kT = _bf16(_pack_acts(kf.transpose(0, 2, 1)))
    vT = _bf16(_pack_acts(vf.transpose(0, 2, 1)))
    wq = _bf16(_pack_w((w_qs / temp).T))
    wk = _bf16(_pack_w(w_ks.T))
    wv = _bf16(_pack_w(w_vs.T))
    # wfc packed as [pt, d_low, dc, p_in_pt]: wfcT_eff[dc*128+d_low, pt*512+p]
    wfcT_eff = (w_fc * ln_gamma[None, :]).T  # [D, P]
    wfc = _bf16(wfcT_eff.reshape(4, 128, NPC, 512).transpose(2, 1, 0, 3))
    # colsum[d_low, dc, pt] = sum_{p in pt} wfcT_eff[dc*128+d_low, p]
    colsum = _bf16(wfcT_eff.reshape(4, 128, NPC, 512).sum(-1).transpose(1, 0, 2))
    bias_fc = (w_fc @ ln_beta).astype(np.float32)
    veff = vf + bias_fc[None, None, :]                      # [B, C, P] f32
    resid = _bf16(veff.reshape(B, C, NPC, 512))
    # per-(b,pt) channel sums of the (bf16-rounded) residual, f32 accumulated
    vsums_full = np.asarray(resid, np.float32).sum(-1)      # [B, C, NPC]
    bng = np.ascontiguousarray(bn_gamma.reshape(C, 1))
    bnb = np.ascontiguousarray(bn_beta.reshape(C, 1))

    nc = _build()
    in_maps = []
    for i in range(N_CORES):
        bs = slice(BPC * i, BPC * (i + 1))
        vsums = np.ascontiguousarray(
            vsums_full[bs].transpose(1, 0, 2).reshape(C, 2 * NPC))
        in_maps.append({
            "qT": qT[bs], "kT": kT[bs], "vT": vT[bs], "resid": resid[bs],
            "wq": wq, "wk": wk, "wv": wv, "wfc": wfc,
            "colsum": colsum, "vsums": vsums,
            "bng": bng, "bnb": bnb,
        })
    res = run_bass_kernel_spmd(nc, in_maps, core_ids=list(range(N_CORES)))
    global LAST_RESULTS
    LAST_RESULTS = res
    out = np.concatenate([np.asarray(res.results[i]["out"], np.float32)
                          for i in range(N_CORES)], axis=0)
    return out.reshape(B, C, HH, WW)


# revision 21
# speedup vs baseline: 1.0456x; 1.0286x over previous
"""Trainium2 Bass kernel for nn_MultiHeadAttention (channel-attention transformer block).

Math (per batch b, with X* = reshape(*, [C, P]), P = 4096, C = 128, D = 512):
  Q = Xq @ (Wq/temp)^T, K = Xk @ Wk^T, V = Xv @ Wv^T            [C, D]
  per head h (8 heads, ld=64): A_h = softmax(Q_h K_h^T); O_h = A_h V_h
  O = silu(O); O = (O - mean)/(unbiased_std + eps)   (LN affine folded into fc)
  out_pre = (v + Wfc@ln_beta) + O @ (Wfc*ln_gamma)^T
  out = BatchNorm2d(out_pre)   (batch stats over (b,h,w), biased var)

Sharding: data-parallel over batch, 2 batches per core on 8 cores; BatchNorm
statistics combined with a tiny AllReduce ([128,2] per core).  Two warm-up
AllReduces (kernel start + mid) absorb launch skew + keep ncfw hot so the
real AllReduce at the end runs near its latency floor.

v3 notes:
 - batch-pipelined: batch0's projections complete first (weights + b0
   activations DMA'd first); b0's attention runs while b1's projections
   stream; b1's attention interleaves with b0's fc phase.
 - single ACT table set (natural_log_exp): silu computed as x*(1/(1+e^-x))
   with DVE ops; LN/BN rstd via exp(-0.5*ln(var)); Square/Copy/Identity are
   table fillers.  Avoids ~2.7us ACT_TABLE_LOAD thrash per switch.
 - residual added into the fc PSUM via an identity-matmul accumulation step.
 - BN per-channel sums from host-precomputed residual sums + xhatT@colsum(wfc)
   matmuls; only sum-of-squares needs Scalar passes.
"""

import os

import numpy as np

import concourse.mybir as mybir
import concourse.tile as tile
from concourse import bacc
from concourse.bass_utils import run_bass_kernel_spmd
from concourse.masks import make_identity

# ---- problem constants (hardcoded per contract) ----
B, C, HH, WW = 16, 128, 64, 64
P = HH * WW           # 4096
NH, LD = 8, 64
D = NH * LD           # 512
N_CORES = 8
BPC = B // N_CORES    # 2 batches per core
NPC = P // 512        # 8 quad-chunks over contraction / output tiles
LN_EPS = 1e-6
BN_EPS = 1e-5
F32 = mybir.dt.float32
BF16 = mybir.dt.bfloat16

_BUILD_CACHE: dict = {}
LAST_RESULTS = None  # BassKernelResults of the most recent run (for profiling)

SKIP_COLL = os.environ.get("BASS_SKIP_COLL", "0") == "1"
N_WARM_AR = int(os.environ.get("BASS_WARM_AR", "2"))
USE_LNEXP = os.environ.get("BASS_LNEXP", "0") == "1"
MODE = "bf16"  # kept for test harness printing


def _emit(ctx, nc, tc, io):
    AF = mybir.ActivationFunctionType
    ALU = mybir.AluOpType
    AX = mybir.AxisListType

    consts = ctx.enter_context(tc.tile_pool(name="consts", bufs=1))
    wpool = ctx.enter_context(tc.tile_pool(name="wpool", bufs=1))
    fcpool = ctx.enter_context(tc.tile_pool(name="fcpool", bufs=1))
    apool = ctx.enter_context(tc.tile_pool(name="apool", bufs=2))
    a1pool = ctx.enter_context(tc.tile_pool(name="a1pool", bufs=3))
    sb = ctx.enter_context(tc.tile_pool(name="sb", bufs=2))
    big = ctx.enter_context(tc.tile_pool(name="big", bufs=1))
    small = ctx.enter_context(tc.tile_pool(name="small", bufs=4))
    stat = ctx.enter_context(tc.tile_pool(name="stat", bufs=1))
    dram = ctx.enter_context(tc.tile_pool(name="dram", bufs=1, space="DRAM"))

    # identity for PE transposes / residual accumulation
    ident_f = consts.tile([128, 128], F32, tag="identf", name="identf")
    make_identity(nc, ident_f)
    ident = consts.tile([128, 128], BF16, tag="ident", name="ident")
    nc.vector.tensor_copy(out=ident, in_=ident_f)

    # ---- warm-up AllReduces: absorb launch skew + keep ncfw hot ----
    if not SKIP_COLL and N_WARM_AR > 0:
        wsb = consts.tile([128, 2], F32, tag="wsb", name="wsb")
        nc.vector.memset(wsb, 0.0)
        cwin = dram.tile([128, 2], F32, tag="cwin", name="cwin")
        cwout = dram.tile([128, 2], F32, tag="cwout", name="cwout")
        nc.gpsimd.dma_start(out=cwin[:, :], in_=wsb)
        for _ in range(N_WARM_AR):
            nc.gpsimd.collective_compute(
                "AllReduce",
                ALU.add,
                replica_groups=[list(range(N_CORES))],
                ins=[cwin.opt()],
                outs=[cwout.opt()],
            )

    # ================= phase A0: weights + batch0 projections =================
    ps_proj0_ctx = tc.tile_pool(name="ps_proj0", bufs=1, space="PSUM")
    ps_proj0 = ps_proj0_ctx.__enter__()
    warm = ps_proj0.tile([128, 128], BF16, tag="warm", name="warm")
    nc.tensor.transpose(warm[:, :], ident[:, :], ident[:, :])
    Qp0 = ps_proj0.tile([128, D], F32, tag="Qp0", name="Qp0")
    Kp0 = ps_proj0.tile([128, D], F32, tag="Kp0", name="Kp0")
    Vp0 = ps_proj0.tile([128, D], F32, tag="Vp0", name="Vp0")

    wts = []  # held weight chunk tiles, reused for batch1
    for pc in range(NPC):
        wq_c = wpool.tile([128, 4, D], BF16, tag=f"wq{pc}", name=f"wq{pc}")
        wk_c = wpool.tile([128, 4, D], BF16, tag=f"wk{pc}", name=f"wk{pc}")
        wv_c = wpool.tile([128, 4, D], BF16, tag=f"wv{pc}", name=f"wv{pc}")
        nc.sync.dma_start(out=wq_c, in_=io["wq"][pc])
        nc.scalar.dma_start(out=wk_c, in_=io["wk"][pc])
        nc.gpsimd.dma_start(out=wv_c, in_=io["wv"][pc])
        wts.append((wq_c, wk_c, wv_c))
        qc = apool.tile([128, 4, 128], BF16, tag="qc0", name="qc0")
        kc = apool.tile([128, 4, 128], BF16, tag="kc0", name="kc0")
        vc = apool.tile([128, 4, 128], BF16, tag="vc0", name="vc0")
        nc.sync.dma_start(out=qc, in_=io["qT"][0, pc])
        nc.scalar.dma_start(out=kc, in_=io["kT"][0, pc])
        nc.gpsimd.dma_start(out=vc, in_=io["vT"][0, pc])
        for j in range(4):
            st = pc == 0 and j == 0
            sp = pc == NPC - 1 and j == 3
            nc.tensor.matmul(Qp0[:, :], qc[:, j, :], wq_c[:, j, :], start=st, stop=sp)
            nc.tensor.matmul(Kp0[:, :], kc[:, j, :], wk_c[:, j, :], start=st, stop=sp)
            nc.tensor.matmul(Vp0[:, :], vc[:, j, :], wv_c[:, j, :], start=st, stop=sp)

    # batch1 activation streams (behind phase-A0 transfers on the same queues)
    a1 = []
    for pc in range(NPC):
        qc = a1pool.tile([128, 4, 128], BF16, tag="qc1", name="qc1")
        kc = a1pool.tile([128, 4, 128], BF16, tag="kc1", name="kc1")
        vc = a1pool.tile([128, 4, 128], BF16, tag="vc1", name="vc1")
        nc.sync.dma_start(out=qc, in_=io["qT"][1, pc])
        nc.scalar.dma_start(out=kc, in_=io["kT"][1, pc])
        nc.gpsimd.dma_start(out=vc, in_=io["vT"][1, pc])
        a1.append((qc, kc, vc))

    # fc weights / residual / stats constants stream during the attention era
    wfcts = []
    for pt in range(NPC):
        wfct = fcpool.tile([128, 4, 512], BF16, tag=f"wfct{pt}", name=f"wfct{pt}")
        (nc.sync if pt % 2 == 0 else nc.gpsimd).dma_start(out=wfct, in_=io["wfc"][pt])
        wfcts.append(wfct)
    resid = []
    for b in range(BPC):
        t = fcpool.tile([128, NPC, 512], BF16, tag=f"resid{b}", name=f"resid{b}")
        nc.scalar.dma_start(out=t, in_=io["resid"][b])
        resid.append(t)
    colsum = consts.tile([128, 4, NPC], BF16, tag="colsum", name="colsum")
    nc.gpsimd.dma_start(out=colsum, in_=io["colsum"][:, :, :])
    vsums = consts.tile([128, 2 * NPC], F32, tag="vsums", name="vsums")
    nc.gpsimd.dma_start(out=vsums, in_=io["vsums"][:, :])
    bng = consts.tile([128, 1], F32, tag="bng", name="bng")
    bnb = consts.tile([128, 1], F32, tag="bnb", name="bnb")
    nc.gpsimd.dma_start(out=bng, in_=io["bng"][:, :])
    nc.gpsimd.dma_start(out=bnb, in_=io["bnb"][:, :])

    # ---- evacuate batch0 Q/K/V, free proj0 banks ----
    Q_sb0 = sb.tile([128, D], BF16, tag="Q_sb", name="Q_sb0")
    K_sb0 = sb.tile([128, D], BF16, tag="K_sb", name="K_sb0")
    V_sb0 = sb.tile([128, D], BF16, tag="V_sb", name="V_sb0")
    nc.vector.tensor_copy(out=Q_sb0, in_=Qp0[:, :])
    nc.scalar.copy(out=K_sb0, in_=Kp0[:, :])
    nc.vector.tensor_copy(out=V_sb0, in_=Vp0[:, :])
    ps_proj0_ctx.__exit__(None, None, None)

    # attention-era PSUM (sm x3 + O0/O1) + batch1 projection banks (x3) = 8
    ps_s = ctx.enter_context(tc.tile_pool(name="ps_s", bufs=3, space="PSUM"))
    ps_o = ctx.enter_context(tc.tile_pool(name="ps_o", bufs=1, space="PSUM"))
    ps_proj1_ctx = tc.tile_pool(name="ps_proj1", bufs=1, space="PSUM")
    ps_proj1 = ps_proj1_ctx.__enter__()
    Qp1 = ps_proj1.tile([128, D], F32, tag="Qp1", name="Qp1")
    Kp1 = ps_proj1.tile([128, D], F32, tag="Kp1", name="Kp1")
    Vp1 = ps_proj1.tile([128, D], F32, tag="Vp1", name="Vp1")

    Opsums = [ps_o.tile([128, D], F32, tag=f"O{b}", name=f"O{b}") for b in range(BPC)]
    Oscs = [big.tile([128, D], F32, tag=f"Osc{b}", name=f"Osc{b}") for b in range(BPC)]

    def qk_transposes(Q_sb, K_sb, QT_sb, KT_sb):
        for si, (src, dst) in enumerate(((Q_sb, QT_sb), (K_sb, KT_sb))):
            for dc in range(4):
                tp = ps_s.tile([128, 128], BF16, tag="sm", name="stp")
                nc.tensor.transpose(tp[:, :], src[:, dc * 128:(dc + 1) * 128], ident[:, :])
                if (si + dc) % 2 == 0:
                    nc.vector.tensor_copy(out=dst[:, dc * 128:(dc + 1) * 128], in_=tp[:, :])
                else:
                    nc.scalar.copy(out=dst[:, dc * 128:(dc + 1) * 128], in_=tp[:, :])

    def attn_head(b, h, QT_sb, KT_sb, V_sb, filler):
        """One attention head; `filler` emits PE work into the Exp-wait gap."""
        po = (h % 2) * 64
        fo = (h // 2) * 128
        S = ps_s.tile([128, 128], F32, tag="sm", name="S")
        nc.tensor.matmul(S[:, :], QT_sb[po:po + 64, fo:fo + 128],
                         KT_sb[po:po + 64, fo:fo + 128], start=True, stop=True)
        e_f = sb.tile([128, 128], BF16, tag="e_f", name="e_f")
        lsum = small.tile([128, 1], F32, tag="lsum", name="lsum")
        nc.scalar.activation(out=e_f, in_=S[:, :], func=AF.Exp, accum_out=lsum)
        rs = small.tile([128, 1], F32, tag="rs", name="rs")
        nc.vector.reciprocal(rs, lsum)
        if filler is not None:
            filler(h)
        tpa = ps_s.tile([128, 128], BF16, tag="sm", name="stp")
        nc.tensor.transpose(tpa[:, :], e_f[:, :], ident[:, :])
        aT = sb.tile([128, 128], BF16, tag="aT", name="aT")
        nc.vector.tensor_copy(out=aT, in_=tpa[:, :])
        nc.tensor.matmul(Opsums[b][:, h * 64:(h + 1) * 64], aT[:, :],
                         V_sb[:, h * 64:(h + 1) * 64], start=True, stop=True)
        nc.vector.tensor_scalar_mul(out=Oscs[b][:, h * 64:(h + 1) * 64],
                                    in0=Opsums[b][:, h * 64:(h + 1) * 64],
                                    scalar1=rs)

    # ---- batch0 attention, with batch1 projections as PE filler ----
    QT0 = sb.tile([128, D], BF16, tag="QT_sb", name="QT0")
    KT0 = sb.tile([128, D], BF16, tag="KT_sb", name="KT0")
    qk_transposes(Q_sb0, K_sb0, QT0, KT0)

    def proj1_filler(h):
        qc, kc, vc = a1[h]
        wq_c, wk_c, wv_c = wts[h]
        for j in range(4):
            st = h == 0 and j == 0
            sp = h == NPC - 1 and j == 3
            nc.tensor.matmul(Qp1[:, :], qc[:, j, :], wq_c[:, j, :], start=st, stop=sp)
            nc.tensor.matmul(Kp1[:, :], kc[:, j, :], wk_c[:, j, :], start=st, stop=sp)
            nc.tensor.matmul(Vp1[:, :], vc[:, j, :], wv_c[:, j, :], start=st, stop=sp)

    for h in range(NH):
        attn_head(0, h, QT0, KT0, V_sb0, proj1_filler)

    # ---- evacuate batch1 Q/K/V, free proj1 banks; open fc-era pools ----
    Q_sb1 = sb.tile([128, D], BF16, tag="Q_sb", name="Q_sb1")
    K_sb1 = sb.tile([128, D], BF16, tag="K_sb", name="K_sb1")
    V_sb1 = sb.tile([128, D], BF16, tag="V_sb", name="V_sb1")
    nc.vector.tensor_copy(out=Q_sb1, in_=Qp1[:, :])
    nc.scalar.copy(out=K_sb1, in_=Kp1[:, :])
    nc.vector.tensor_copy(out=V_sb1, in_=Vp1[:, :])
    ps_proj1_ctx.__exit__(None, None, None)
    ps_fc = ctx.enter_context(tc.tile_pool(name="ps_fc", bufs=2, space="PSUM"))
    ps_fs = ctx.enter_context(tc.tile_pool(name="ps_fs", bufs=1, space="PSUM"))
    fcS = ps_fs.tile([128, 2 * NPC], F32, tag="fcS", name="fcS")

    def silu_ln_xt(b):
        """silu (exp + DVE) -> LayerNorm -> xhat^T; returns xT tile."""
        Osc = Oscs[b]
        e1 = big.tile([128, D], F32, tag="e1", name="e1", bufs=2)
        nc.scalar.activation(out=e1, in_=Osc, func=AF.Exp, scale=-1.0)
        nc.vector.tensor_scalar_add(out=e1, in0=e1, scalar1=1.0)
        nc.vector.reciprocal(e1, e1)
        nc.vector.tensor_mul(out=Osc, in0=Osc, in1=e1)   # Osc <- silu(Osc)
        st6 = small.tile([128, 6], F32, tag="st6", name="st6")
        nc.vector.bn_stats(out=st6, in_=Osc)
        mv = small.tile([128, 2], F32, tag="mv", name="mv")
        nc.vector.bn_aggr(out=mv, in_=st6)
        rstd = small.tile([128, 1], F32, tag="rstd", name="rstd")
        if USE_LNEXP:
            # 1/(sqrt(v)+eps) ~= exp(-0.5*ln(v)) (eps=1e-6 negligible);
            # stays in the natural_log_exp ACT table set
            lt = small.tile([128, 1], F32, tag="lt", name="lt")
            nc.scalar.activation(out=lt, in_=mv[:, 1:2], func=AF.Ln,
                                 scale=float(D) / (D - 1))
            nc.scalar.activation(out=rstd, in_=lt, func=AF.Exp, scale=-0.5)
        else:
            sd = small.tile([128, 1], F32, tag="sd", name="sd")
            nc.scalar.activation(out=sd, in_=mv[:, 1:2], func=AF.Sqrt,
                                 scale=float(D) / (D - 1))
            nc.vector.tensor_scalar_add(out=sd, in0=sd, scalar1=LN_EPS)
            nc.vector.reciprocal(rstd, sd)
        xhat = sb.tile([128, D], BF16, tag="xhat", name="xhat")
        nc.vector.tensor_scalar(out=xhat, in0=Osc, scalar1=mv[:, 0:1], scalar2=rstd,
                                op0=ALU.subtract, op1=ALU.mult)
        xT = sb.tile([128, D], BF16, tag="xT", name="xT")
        for dc in range(4):
            tp = ps_s.tile([128, 128], BF16, tag="sm", name="stp")
            nc.tensor.transpose(tp[:, :], xhat[:, dc * 128:(dc + 1) * 128], ident[:, :])
            if dc % 2 == 0:
                nc.vector.tensor_copy(out=xT[:, dc * 128:(dc + 1) * 128], in_=tp[:, :])
            else:
                nc.scalar.copy(out=xT[:, dc * 128:(dc + 1) * 128], in_=tp[:, :])
        for dc in range(4):
            nc.tensor.matmul(fcS[:, b * NPC:(b + 1) * NPC],
                             xT[:, dc * 128:(dc + 1) * 128],
                             colsum[:, dc, :], start=dc == 0, stop=dc == 3)
        return xT

    segs = [big.tile([128, NPC, 512], BF16, tag=f"seg{b}", name=f"seg{b}")
            for b in range(BPC)]
    pcols = stat.tile([128, 16], F32, tag="pcols", name="pcols")

    def fc_pt(b, pt, xT):
        """fc for one 512-pixel tile + residual + sumsq accumulation."""
        O2 = ps_fc.tile([128, 512], F32, tag="O2", name="O2")
        for dc in range(4):
            nc.tensor.matmul(O2[:, :], xT[:, dc * 128:(dc + 1) * 128],
                             wfcts[pt][:, dc, :], start=dc == 0, stop=False)
        nc.tensor.matmul(O2[:, :], ident[:, :], resid[b][:, pt, :],
                         start=False, stop=True)
        seg = segs[b][:, pt, :]
        if pt % 2 == 0:
            nc.vector.tensor_copy(out=seg, in_=O2[:, :])
        else:
            nc.scalar.copy(out=seg, in_=O2[:, :])
        junk = sb.tile([128, 512], BF16, tag="junk", name="junk")
        nc.scalar.activation(out=junk, in_=O2[:, :], func=AF.Square,
                             accum_out=pcols[:, b * NPC + pt:b * NPC + pt + 1])

    # ---- batch0 silu/LN/xT, then batch1 attention with batch0 fc as filler ----
    xT0 = silu_ln_xt(0)
    QT1 = sb.tile([128, D], BF16, tag="QT_sb", name="QT1")
    KT1 = sb.tile([128, D], BF16, tag="KT_sb", name="KT1")
    qk_transposes(Q_sb1, K_sb1, QT1, KT1)

    def fc0_filler(h):
        fc_pt(0, h, xT0)

    for h in range(NH):
        attn_head(1, h, QT1, KT1, V_sb1, fc0_filler)

    # ---- batch1 silu/LN/xT + fc ----
    xT1 = silu_ln_xt(1)
    for pt in range(NPC):
        fc_pt(1, pt, xT1)

    # ---- BN stats AllReduce + normalize + store ----
    stats2 = stat.tile([128, 2], F32, tag="stats2", name="stats2")
    ssum = stat.tile([128, NPC], F32, tag="ssum", name="ssum")
    nc.vector.tensor_add(out=ssum, in0=vsums[:, 0:NPC], in1=vsums[:, NPC:2 * NPC])
    nc.vector.tensor_add(out=ssum, in0=ssum, in1=fcS[:, 0:NPC])
    nc.vector.tensor_add(out=ssum, in0=ssum, in1=fcS[:, NPC:2 * NPC])
    nc.vector.reduce_sum(stats2[:, 0:1], ssum, axis=AX.X)
    nc.vector.reduce_sum(stats2[:, 1:2], pcols[:, 0:16], axis=AX.X)

    cin = dram.tile([128, 2], F32, tag="cin", name="cin")
    cout = dram.tile([128, 2], F32, tag="cout", name="cout")
    nc.gpsimd.dma_start(out=cin[:, :], in_=stats2)
    if SKIP_COLL:
        nc.gpsimd.dma_start(out=cout[:, :], in_=cin[:, :])
    else:
        nc.gpsimd.collective_compute(
            "AllReduce",
            ALU.add,
            replica_groups=[list(range(N_CORES))],
            ins=[cin.opt()],
            outs=[cout.opt()],
        )
    red = stat.tile([128, 2], F32, tag="red", name="red")
    nc.gpsimd.dma_start(out=red[:, :], in_=cout[:, :])

    inv_n = 1.0 / float(B * P)
    epsbn = consts.tile([128, 1], F32, tag="epsbn", name="epsbn")
    nc.vector.memset(epsbn, BN_EPS)
    me = stat.tile([128, 2], F32, tag="me", name="me")  # [mean, E[x^2]]
    nc.vector.tensor_scalar_mul(out=me, in0=red[:, :], scalar1=inv_n)
    msq = small.tile([128, 1], F32, tag="msq", name="msq")
    nc.vector.tensor_mul(out=msq, in0=me[:, 0:1], in1=me[:, 0:1])
    var = small.tile([128, 1], F32, tag="var", name="var")
    nc.vector.tensor_sub(out=var, in0=me[:, 1:2], in1=msq)
    invs = small.tile([128, 1], F32, tag="invs", name="invs")
    if USE_LNEXP:
        lt2 = small.tile([128, 1], F32, tag="lt2", name="lt2")
        nc.scalar.activation(out=lt2, in_=var, func=AF.Ln, bias=epsbn)
        nc.scalar.activation(out=invs, in_=lt2, func=AF.Exp, scale=-0.5)
    else:
        sdv = small.tile([128, 1], F32, tag="sdv", name="sdv")
        nc.scalar.activation(out=sdv, in_=var, func=AF.Sqrt, bias=epsbn)
        nc.vector.reciprocal(invs, sdv)
    scl = small.tile([128, 1], F32, tag="scl", name="scl")
    nc.vector.tensor_mul(out=scl, in0=bng, in1=invs)
    tmp = small.tile([128, 1], F32, tag="tmp", name="tmp")
    nc.vector.tensor_mul(out=tmp, in0=me[:, 0:1], in1=scl)
    shf = small.tile([128, 1], F32, tag="shf", name="shf")
    nc.vector.tensor_sub(out=shf, in0=bnb, in1=tmp)

    dq = [nc.gpsimd, nc.sync, nc.scalar]
    for pt in range(NPC):
        for b in range(BPC):
            seg = segs[b][:, pt, :]
            if (pt + b) % 2 == 0:
                nc.vector.tensor_scalar(out=seg, in0=seg, scalar1=scl, scalar2=shf,
                                        op0=ALU.mult, op1=ALU.add)
            else:
                nc.scalar.activation(out=seg, in_=seg, func=AF.Identity,
                                     scale=scl, bias=shf)
            dq[(pt * BPC + b) % 3].dma_start(out=io["out"][b, :, pt * 512:(pt + 1) * 512],
                                             in_=seg)


def _build():
    key = (SKIP_COLL, N_WARM_AR, USE_LNEXP)
    if key in _BUILD_CACHE:
        return _BUILD_CACHE[key]
    nc = bacc.Bacc("TRN2", target_bir_lowering=False, debug=False, num_devices=N_CORES)
    io = {
        "qT": nc.dram_tensor("qT", [BPC, NPC, 128, 4, 128], BF16, kind="ExternalInput").ap(),
        "kT": nc.dram_tensor("kT", [BPC, NPC, 128, 4, 128], BF16, kind="ExternalInput").ap(),
        "vT": nc.dram_tensor("vT", [BPC, NPC, 128, 4, 128], BF16, kind="ExternalInput").ap(),
        "resid": nc.dram_tensor("resid", [BPC, C, NPC, 512], BF16, kind="ExternalInput").ap(),
        "wq": nc.dram_tensor("wq", [NPC, 128, 4, D], BF16, kind="ExternalInput").ap(),
        "wk": nc.dram_tensor("wk", [NPC, 128, 4, D], BF16, kind="ExternalInput").ap(),
        "wv": nc.dram_tensor("wv", [NPC, 128, 4, D], BF16, kind="ExternalInput").ap(),
        "wfc": nc.dram_tensor("wfc", [NPC, 128, 4, 512], BF16, kind="ExternalInput").ap(),
        "colsum": nc.dram_tensor("colsum", [128, 4, NPC], BF16, kind="ExternalInput").ap(),
        "vsums": nc.dram_tensor("vsums", [C, 2 * NPC], F32, kind="ExternalInput").ap(),
        "bng": nc.dram_tensor("bng", [C, 1], F32, kind="ExternalInput").ap(),
        "bnb": nc.dram_tensor("bnb", [C, 1], F32, kind="ExternalInput").ap(),
        "out": nc.dram_tensor("out", [BPC, C, P], BF16, kind="ExternalOutput").ap(),
    }
    from contextlib import ExitStack
    with tile.TileContext(nc) as tc, ExitStack() as ctx:
        _emit(ctx, nc, tc, io)
    nc.compile()
    _BUILD_CACHE[key] = nc
    return nc


def _bf16(x):
    import ml_dtypes
    return np.ascontiguousarray(np.asarray(x, np.float32).astype(ml_dtypes.bfloat16))


def _pack_acts(xT):
    # [b, 4096, 128] -> [b, NPC, 128, 4, 128]  (pc-chunk, partition, j, c)
    b = xT.shape[0]
    return np.ascontiguousarray(
        xT.reshape(b, NPC, 4, 128, 128).transpose(0, 1, 3, 2, 4))


def _pack_w(w):
    # [4096, D] -> [NPC, 128, 4, D]
    return np.ascontiguousarray(w.reshape(NPC, 4, 128, -1).transpose(0, 2, 1, 3))


def kernel(v, k, q, w_qs, w_ks, w_vs, w_fc, ln_gamma, ln_beta, temperature,
           bn_gamma, bn_beta, **_ignored):
    v = np.asarray(v, np.float32)
    k = np.asarray(k, np.float32)
    q = np.asarray(q, np.float32)
    w_qs = np.asarray(w_qs, np.float32)
    w_ks = np.asarray(w_ks, np.float32)
    w_vs = np.asarray(w_vs, np.float32)
    w_fc = np.asarray(w_fc, np.float32)
    ln_gamma = np.asarray(ln_gamma, np.float32)
    ln_beta = np.asarray(ln_beta, np.float32)
    temp = float(np.asarray(temperature))
    bn_gamma = np.asarray(bn_gamma, np.float32)
    bn_beta = np.asarray(bn_beta, np.float32)

    qf = q.reshape(B, C, P)
    kf = k.reshape(B, C, P)
    vf = v.reshape(B, C, P)
    qT = _bf16(_pack_acts(qf.transpose(0, 2, 1)))
    kT = _bf16(_pack_acts(kf.transpose(0, 2, 1)))
    vT = _bf16(_pack_acts(vf.transpose(0, 2, 1)))
    wq = _bf16(_pack_w((w_qs / temp).T))
    wk = _bf16(_pack_w(w_ks.T))
    wv = _bf16(_pack_w(w_vs.T))
    # wfc packed as [pt, d_low, dc, p_in_pt]: wfcT_eff[dc*128+d_low, pt*512+p]
    wfcT_eff = (w_fc * ln_gamma[None, :]).T  # [D, P]
    wfc = _bf16(wfcT_eff.reshape(4, 128, NPC, 512).transpose(2, 1, 0, 3))
    # colsum[d_low, dc, pt] = sum_{p in pt} wfcT_eff[dc*128+d_low, pt*512+p]
    colsum = _bf16(wfcT_eff.reshape(4, 128, NPC, 512).sum(-1).transpose(1, 0, 2))
    bias_fc = (w_fc @ ln_beta).astype(np.float32)
    veff = vf + bias_fc[None, None, :]                      # [B, C, P] f32
    resid = _bf16(veff.reshape(B, C, NPC, 512))
    # per-(b,pt) channel sums of the (bf16-rounded) residual, f32 accumulated
    vsums_full = np.asarray(resid, np.float32).sum(-1)      # [B, C, NPC]
    bng = np.ascontiguousarray(bn_gamma.reshape(C, 1))
    bnb = np.ascontiguousarray(bn_beta.reshape(C, 1))

    nc = _build()
    in_maps = []
    for i in range(N_CORES):
        bs = slice(BPC * i, BPC * (i + 1))
        vsums = np.ascontiguousarray(
            vsums_full[bs].transpose(1, 0, 2).reshape(C, 2 * NPC))
        in_maps.append({
            "qT": qT[bs], "kT": kT[bs], "vT": vT[bs], "resid": resid[bs],
            "wq": wq, "wk": wk, "wv": wv, "wfc": wfc,
            "colsum": colsum, "vsums": vsums,
            "bng": bng, "bnb": bnb,
        })
    res = run_bass_kernel_spmd(nc, in_maps, core_ids=list(range(N_CORES)))
    global LAST_RESULTS
    LAST_RESULTS = res
    out = np.concatenate([np.asarray(res.results[i]["out"], np.float32)
                          for i in range(N_CORES)], axis=0)
    return out.reshape(B, C, HH, WW)


# revision 26
# speedup vs baseline: 1.0785x; 1.0315x over previous
"""Trainium2 Bass kernel for nn_MultiHeadAttention (channel-attention transformer block).

Math (per batch b, with X* = reshape(*, [C, P]), P = 4096, C = 128, D = 512):
  Q = Xq @ (Wq/temp)^T, K = Xk @ Wk^T, V = Xv @ Wv^T            [C, D]
  per head h (8 heads, ld=64): A_h = softmax(Q_h K_h^T); O_h = A_h V_h
  O = silu(O); O = (O - mean)/(unbiased_std + eps)   (LN affine folded into fc)
  out_pre = (v + Wfc@ln_beta) + O @ (Wfc*ln_gamma)^T
  out = BatchNorm2d(out_pre)   (batch stats over (b,h,w), biased var)

Sharding: data-parallel over batch, 2 batches per core on 8 cores; BatchNorm
statistics combined with a tiny AllReduce ([128,2] per core).  Two warm-up
AllReduces (kernel start + mid) absorb launch skew + keep ncfw hot so the
real AllReduce at the end runs near its latency floor.

v3 notes:
 - batch-pipelined: batch0's projections complete first (weights + b0
   activations DMA'd first); b0's attention runs while b1's projections
   stream; b1's attention interleaves with b0's fc phase.
 - single ACT table set (natural_log_exp): silu computed as x*(1/(1+e^-x))
   with DVE ops; LN/BN rstd via exp(-0.5*ln(var)); Square/Copy/Identity are
   table fillers.  Avoids ~2.7us ACT_TABLE_LOAD thrash per switch.
 - residual added into the fc PSUM via an identity-matmul accumulation step.
 - BN per-channel sums from host-precomputed residual sums + xhatT@colsum(wfc)
   matmuls; only sum-of-squares needs Scalar passes.
"""

import os

import numpy as np

import concourse.mybir as mybir
import concourse.tile as tile
from concourse import bacc
from concourse.bass_utils import run_bass_kernel_spmd
from concourse.masks import make_identity

# ---- problem constants (hardcoded per contract) ----
B, C, HH, WW = 16, 128, 64, 64
P = HH * WW           # 4096
NH, LD = 8, 64
D = NH * LD           # 512
N_CORES = 8
BPC = B // N_CORES    # 2 batches per core
NPC = P // 512        # 8 quad-chunks over contraction / output tiles
LN_EPS = 1e-6
BN_EPS = 1e-5
F32 = mybir.dt.float32
BF16 = mybir.dt.bfloat16

_BUILD_CACHE: dict = {}
LAST_RESULTS = None  # BassKernelResults of the most recent run (for profiling)

SKIP_COLL = os.environ.get("BASS_SKIP_COLL", "0") == "1"
N_WARM_AR = int(os.environ.get("BASS_WARM_AR", "2"))
USE_LNEXP = os.environ.get("BASS_LNEXP", "0") == "1"
MODE = "bf16"  # kept for test harness printing


def _emit(ctx, nc, tc, io):
    AF = mybir.ActivationFunctionType
    ALU = mybir.AluOpType
    AX = mybir.AxisListType

    consts = ctx.enter_context(tc.tile_pool(name="consts", bufs=1))
    wpool = ctx.enter_context(tc.tile_pool(name="wpool", bufs=1))
    fcpool = ctx.enter_context(tc.tile_pool(name="fcpool", bufs=1))
    apool = ctx.enter_context(tc.tile_pool(name="apool", bufs=3))
    a1pool = ctx.enter_context(tc.tile_pool(name="a1pool", bufs=3))
    sb = ctx.enter_context(tc.tile_pool(name="sb", bufs=2))
    big = ctx.enter_context(tc.tile_pool(name="big", bufs=1))
    small = ctx.enter_context(tc.tile_pool(name="small", bufs=4))
    stat = ctx.enter_context(tc.tile_pool(name="stat", bufs=1))
    dram = ctx.enter_context(tc.tile_pool(name="dram", bufs=1, space="DRAM"))

    # identity for PE transposes / residual accumulation
    ident_f = consts.tile([128, 128], F32, tag="identf", name="identf")
    make_identity(nc, ident_f)
    ident = consts.tile([128, 128], BF16, tag="ident", name="ident")
    nc.vector.tensor_copy(out=ident, in_=ident_f)

    # ---- warm-up AllReduces: absorb launch skew + keep ncfw hot ----
    if not SKIP_COLL and N_WARM_AR > 0:
        wsb = consts.tile([128, 2], F32, tag="wsb", name="wsb")
        nc.vector.memset(wsb, 0.0)
        cwin = dram.tile([128, 2], F32, tag="cwin", name="cwin")
        cwout = dram.tile([128, 2], F32, tag="cwout", name="cwout")
        nc.gpsimd.dma_start(out=cwin[:, :], in_=wsb)
        for _ in range(N_WARM_AR):
            nc.gpsimd.collective_compute(
                "AllReduce",
                ALU.add,
                replica_groups=[list(range(N_CORES))],
                ins=[cwin.opt()],
                outs=[cwout.opt()],
            )

    # ================= phase A0: weights + batch0 projections =================
    ps_proj0_ctx = tc.tile_pool(name="ps_proj0", bufs=1, space="PSUM")
    ps_proj0 = ps_proj0_ctx.__enter__()
    warm = ps_proj0.tile([128, 128], BF16, tag="warm", name="warm")
    nc.tensor.transpose(warm[:, :], ident[:, :], ident[:, :])
    Qp0 = ps_proj0.tile([128, D], F32, tag="Qp0", name="Qp0")
    Kp0 = ps_proj0.tile([128, D], F32, tag="Kp0", name="Kp0")
    Vp0 = ps_proj0.tile([128, D], F32, tag="Vp0", name="Vp0")

    wts = []  # held weight chunk tiles, reused for batch1
    for pc in range(NPC):
        wq_c = wpool.tile([128, 4, D], BF16, tag=f"wq{pc}", name=f"wq{pc}")
        wk_c = wpool.tile([128, 4, D], BF16, tag=f"wk{pc}", name=f"wk{pc}")
        wv_c = wpool.tile([128, 4, D], BF16, tag=f"wv{pc}", name=f"wv{pc}")
        nc.sync.dma_start(out=wq_c, in_=io["wq"][pc])
        nc.scalar.dma_start(out=wk_c, in_=io["wk"][pc])
        nc.gpsimd.dma_start(out=wv_c, in_=io["wv"][pc])
        wts.append((wq_c, wk_c, wv_c))
        qc = apool.tile([128, 4, 128], BF16, tag="qc0", name="qc0")
        kc = apool.tile([128, 4, 128], BF16, tag="kc0", name="kc0")
        vc = apool.tile([128, 4, 128], BF16, tag="vc0", name="vc0")
        nc.sync.dma_start(out=qc, in_=io["qT"][0, pc])
        nc.scalar.dma_start(out=kc, in_=io["kT"][0, pc])
        nc.gpsimd.dma_start(out=vc, in_=io["vT"][0, pc])
        for j in range(4):
            st = pc == 0 and j == 0
            sp = pc == NPC - 1 and j == 3
            nc.tensor.matmul(Qp0[:, :], qc[:, j, :], wq_c[:, j, :], start=st, stop=sp)
            nc.tensor.matmul(Kp0[:, :], kc[:, j, :], wk_c[:, j, :], start=st, stop=sp)
            nc.tensor.matmul(Vp0[:, :], vc[:, j, :], wv_c[:, j, :], start=st, stop=sp)

    # fc weights / residual / stats constants stream right after phase A0
    # (scalar/gpsimd queues; none of these DMAs carries a WAR wait, so the
    # issuing engines never block on them)
    wfcts = []
    for pt in range(NPC):
        wfct = fcpool.tile([128, 4, 512], BF16, tag=f"wfct{pt}", name=f"wfct{pt}")
        (nc.scalar if pt % 2 == 0 else nc.gpsimd).dma_start(out=wfct, in_=io["wfc"][pt])
        wfcts.append(wfct)
    resid = []
    for b in range(BPC):
        t = fcpool.tile([128, NPC, 512], BF16, tag=f"resid{b}", name=f"resid{b}")
        (nc.scalar if b == 0 else nc.gpsimd).dma_start(out=t, in_=io["resid"][b])
        resid.append(t)
    colsum = consts.tile([128, 4, NPC], BF16, tag="colsum", name="colsum")
    nc.gpsimd.dma_start(out=colsum, in_=io["colsum"][:, :, :])
    vsums = consts.tile([128, 2 * NPC], F32, tag="vsums", name="vsums")
    nc.gpsimd.dma_start(out=vsums, in_=io["vsums"][:, :])
    bng = consts.tile([128, 1], F32, tag="bng", name="bng")
    bnb = consts.tile([128, 1], F32, tag="bnb", name="bnb")
    nc.gpsimd.dma_start(out=bng, in_=io["bng"][:, :])
    nc.gpsimd.dma_start(out=bnb, in_=io["bnb"][:, :])

    # batch1 activation streams: all on the sync queue, after the prefetches.
    # The a1pool rotation gives the later descgens WAR waits; sync's only
    # later work is the output stores, so the blocking is harmless there.
    a1 = []
    for pc in range(NPC):
        qc = a1pool.tile([128, 4, 128], BF16, tag="qc1", name="qc1")
        kc = a1pool.tile([128, 4, 128], BF16, tag="kc1", name="kc1")
        vc = a1pool.tile([128, 4, 128], BF16, tag="vc1", name="vc1")
        nc.sync.dma_start(out=qc, in_=io["qT"][1, pc])
        nc.sync.dma_start(out=kc, in_=io["kT"][1, pc])
        nc.sync.dma_start(out=vc, in_=io["vT"][1, pc])
        a1.append((qc, kc, vc))

    # ---- evacuate batch0 Q/K/V, free proj0 banks ----
    Q_sb0 = sb.tile([128, D], BF16, tag="Q_sb", name="Q_sb0")
    K_sb0 = sb.tile([128, D], BF16, tag="K_sb", name="K_sb0")
    V_sb0 = sb.tile([128, D], BF16, tag="V_sb", name="V_sb0")
    nc.vector.tensor_copy(out=Q_sb0, in_=Qp0[:, :])
    nc.scalar.copy(out=K_sb0, in_=Kp0[:, :])
    nc.vector.tensor_copy(out=V_sb0, in_=Vp0[:, :])
    ps_proj0_ctx.__exit__(None, None, None)

    # attention-era PSUM (sm x3 + O0/O1) + batch1 projection banks (x3) = 8
    ps_s = ctx.enter_context(tc.tile_pool(name="ps_s", bufs=3, space="PSUM"))
    ps_o = ctx.enter_context(tc.tile_pool(name="ps_o", bufs=1, space="PSUM"))
    ps_proj1_ctx = tc.tile_pool(name="ps_proj1", bufs=1, space="PSUM")
    ps_proj1 = ps_proj1_ctx.__enter__()
    Qp1 = ps_proj1.tile([128, D], F32, tag="Qp1", name="Qp1")
    Kp1 = ps_proj1.tile([128, D], F32, tag="Kp1", name="Kp1")
    Vp1 = ps_proj1.tile([128, D], F32, tag="Vp1", name="Vp1")

    Opsums = [ps_o.tile([128, D], F32, tag=f"O{b}", name=f"O{b}") for b in range(BPC)]
    Oscs = [big.tile([128, D], F32, tag=f"Osc{b}", name=f"Osc{b}") for b in range(BPC)]

    def qk_transposes(Q_sb, K_sb, QT_sb, KT_sb):
        # copies all on Vector: Scalar's queue carries Exp/silu chains whose
        # latency must not gate the next batch's S-matmuls
        for src, dst in ((Q_sb, QT_sb), (K_sb, KT_sb)):
            for dc in range(4):
                tp = ps_s.tile([128, 128], BF16, tag="sm", name="stp")
                nc.tensor.transpose(tp[:, :], src[:, dc * 128:(dc + 1) * 128], ident[:, :])
                nc.vector.tensor_copy(out=dst[:, dc * 128:(dc + 1) * 128], in_=tp[:, :])

    def attn_head(b, h, QT_sb, KT_sb, V_sb, filler):
        """One attention head; `filler` emits PE work into the Exp-wait gap."""
        po = (h % 2) * 64
        fo = (h // 2) * 128
        S = ps_s.tile([128, 128], F32, tag="sm", name="S")
        nc.tensor.matmul(S[:, :], QT_sb[po:po + 64, fo:fo + 128],
                         KT_sb[po:po + 64, fo:fo + 128], start=True, stop=True)
        e_f = sb.tile([128, 128], BF16, tag="e_f", name="e_f")
        lsum = small.tile([128, 1], F32, tag="lsum", name="lsum")
        nc.scalar.activation(out=e_f, in_=S[:, :], func=AF.Exp, accum_out=lsum)
        rs = small.tile([128, 1], F32, tag="rs", name="rs")
        nc.vector.reciprocal(rs, lsum)
        if filler is not None:
            filler(h)
        tpa = ps_s.tile([128, 128], BF16, tag="sm", name="stp")
        nc.tensor.transpose(tpa[:, :], e_f[:, :], ident[:, :])
        aT = sb.tile([128, 128], BF16, tag="aT", name="aT")
        nc.vector.tensor_copy(out=aT, in_=tpa[:, :])
        nc.tensor.matmul(Opsums[b][:, h * 64:(h + 1) * 64], aT[:, :],
                         V_sb[:, h * 64:(h + 1) * 64], start=True, stop=True)
        nc.vector.tensor_scalar_mul(out=Oscs[b][:, h * 64:(h + 1) * 64],
                                    in0=Opsums[b][:, h * 64:(h + 1) * 64],
                                    scalar1=rs)

    # ---- batch0 attention, with batch1 projections as PE filler ----
    QT0 = sb.tile([128, D], BF16, tag="QT_sb", name="QT0")
    KT0 = sb.tile([128, D], BF16, tag="KT_sb", name="KT0")
    qk_transposes(Q_sb0, K_sb0, QT0, KT0)

    def proj1_filler(h):
        qc, kc, vc = a1[h]
        wq_c, wk_c, wv_c = wts[h]
        for j in range(4):
            st = h == 0 and j == 0
            sp = h == NPC - 1 and j == 3
            nc.tensor.matmul(Qp1[:, :], qc[:, j, :], wq_c[:, j, :], start=st, stop=sp)
            nc.tensor.matmul(Kp1[:, :], kc[:, j, :], wk_c[:, j, :], start=st, stop=sp)
            nc.tensor.matmul(Vp1[:, :], vc[:, j, :], wv_c[:, j, :], start=st, stop=sp)

    for h in range(NH):
        attn_head(0, h, QT0, KT0, V_sb0, proj1_filler)

    # ---- evacuate batch1 Q/K/V, free proj1 banks; open fc-era pools ----
    Q_sb1 = sb.tile([128, D], BF16, tag="Q_sb", name="Q_sb1")
    K_sb1 = sb.tile([128, D], BF16, tag="K_sb", name="K_sb1")
    V_sb1 = sb.tile([128, D], BF16, tag="V_sb", name="V_sb1")
    nc.vector.tensor_copy(out=Q_sb1, in_=Qp1[:, :])
    nc.scalar.copy(out=K_sb1, in_=Kp1[:, :])
    nc.vector.tensor_copy(out=V_sb1, in_=Vp1[:, :])
    ps_proj1_ctx.__exit__(None, None, None)
    ps_fc = ctx.enter_context(tc.tile_pool(name="ps_fc", bufs=2, space="PSUM"))
    ps_fs = ctx.enter_context(tc.tile_pool(name="ps_fs", bufs=1, space="PSUM"))
    fcS = ps_fs.tile([128, 2 * NPC], F32, tag="fcS", name="fcS")

    def silu_ln_xt(b):
        """silu (exp + DVE) -> LayerNorm -> xhat^T; returns xT tile."""
        Osc = Oscs[b]
        e1 = big.tile([128, D], F32, tag="e1", name="e1", bufs=2)
        nc.scalar.activation(out=e1, in_=Osc, func=AF.Exp, scale=-1.0)
        nc.vector.tensor_scalar_add(out=e1, in0=e1, scalar1=1.0)
        nc.vector.reciprocal(e1, e1)
        nc.vector.tensor_mul(out=Osc, in0=Osc, in1=e1)   # Osc <- silu(Osc)
        st6 = small.tile([128, 6], F32, tag="st6", name="st6")
        nc.vector.bn_stats(out=st6, in_=Osc)
        mv = small.tile([128, 2], F32, tag="mv", name="mv")
        nc.vector.bn_aggr(out=mv, in_=st6)
        rstd = small.tile([128, 1], F32, tag="rstd", name="rstd")
        if USE_LNEXP:
            # 1/(sqrt(v)+eps) ~= exp(-0.5*ln(v)) (eps=1e-6 negligible);
            # stays in the natural_log_exp ACT table set
            lt = small.tile([128, 1], F32, tag="lt", name="lt")
            nc.scalar.activation(out=lt, in_=mv[:, 1:2], func=AF.Ln,
                                 scale=float(D) / (D - 1))
            nc.scalar.activation(out=rstd, in_=lt, func=AF.Exp, scale=-0.5)
        else:
            sd = small.tile([128, 1], F32, tag="sd", name="sd")
            nc.scalar.activation(out=sd, in_=mv[:, 1:2], func=AF.Sqrt,
                                 scale=float(D) / (D - 1))
            nc.vector.tensor_scalar_add(out=sd, in0=sd, scalar1=LN_EPS)
            nc.vector.reciprocal(rstd, sd)
        xhat = sb.tile([128, D], BF16, tag="xhat", name="xhat")
        nc.vector.tensor_scalar(out=xhat, in0=Osc, scalar1=mv[:, 0:1], scalar2=rstd,
                                op0=ALU.subtract, op1=ALU.mult)
        xT = sb.tile([128, D], BF16, tag="xT", name="xT")
        for dc in range(4):
            tp = ps_s.tile([128, 128], BF16, tag="sm", name="stp")
            nc.tensor.transpose(tp[:, :], xhat[:, dc * 128:(dc + 1) * 128], ident[:, :])
            nc.vector.tensor_copy(out=xT[:, dc * 128:(dc + 1) * 128], in_=tp[:, :])
        for dc in range(4):
            nc.tensor.matmul(fcS[:, b * NPC:(b + 1) * NPC],
                             xT[:, dc * 128:(dc + 1) * 128],
                             colsum[:, dc, :], start=dc == 0, stop=dc == 3)
        return xT

    segs = [big.tile([128, NPC, 512], BF16, tag=f"seg{b}", name=f"seg{b}")
            for b in range(BPC)]
    pcols = stat.tile([128, 16], F32, tag="pcols", name="pcols")

    def fc_pt(b, pt, xT):
        """fc for one 512-pixel tile + residual + sumsq accumulation."""
        O2 = ps_fc.tile([128, 512], F32, tag="O2", name="O2")
        for dc in range(4):
            nc.tensor.matmul(O2[:, :], xT[:, dc * 128:(dc + 1) * 128],
                             wfcts[pt][:, dc, :], start=dc == 0, stop=False)
        nc.tensor.matmul(O2[:, :], ident[:, :], resid[b][:, pt, :],
                         start=False, stop=True)
        seg = segs[b][:, pt, :]
        if pt % 2 == 0:
            nc.vector.tensor_copy(out=seg, in_=O2[:, :])
        else:
            nc.scalar.copy(out=seg, in_=O2[:, :])
        junk = sb.tile([128, 512], BF16, tag="junk", name="junk")
        nc.scalar.activation(out=junk, in_=O2[:, :], func=AF.Square,
                             accum_out=pcols[:, b * NPC + pt:b * NPC + pt + 1])

    # ---- batch0 silu/LN/xT, then batch1 attention with batch0 fc as filler ----
    xT0 = silu_ln_xt(0)
    QT1 = sb.tile([128, D], BF16, tag="QT_sb", name="QT1")
    KT1 = sb.tile([128, D], BF16, tag="KT_sb", name="KT1")
    qk_transposes(Q_sb1, K_sb1, QT1, KT1)

    def fc0_filler(h):
        fc_pt(0, h, xT0)

    for h in range(NH):
        attn_head(1, h, QT1, KT1, V_sb1, fc0_filler)

    # ---- batch1 silu/LN/xT + fc ----
    xT1 = silu_ln_xt(1)
    for pt in range(NPC):
        fc_pt(1, pt, xT1)

    # ---- BN stats AllReduce + normalize + store ----
    stats2 = stat.tile([128, 2], F32, tag="stats2", name="stats2")
    ssum = stat.tile([128, NPC], F32, tag="ssum", name="ssum")
    nc.vector.tensor_add(out=ssum, in0=vsums[:, 0:NPC], in1=vsums[:, NPC:2 * NPC])
    nc.vector.tensor_add(out=ssum, in0=ssum, in1=fcS[:, 0:NPC])
    nc.vector.tensor_add(out=ssum, in0=ssum, in1=fcS[:, NPC:2 * NPC])
    nc.vector.reduce_sum(stats2[:, 0:1], ssum, axis=AX.X)
    nc.vector.reduce_sum(stats2[:, 1:2], pcols[:, 0:16], axis=AX.X)

    cin = dram.tile([128, 2], F32, tag="cin", name="cin")
    cout = dram.tile([128, 2], F32, tag="cout", name="cout")
    nc.gpsimd.dma_start(out=cin[:, :], in_=stats2)
    if SKIP_COLL:
        nc.gpsimd.dma_start(out=cout[:, :], in_=cin[:, :])
    else:
        nc.gpsimd.collective_compute(
            "AllReduce",
            ALU.add,
            replica_groups=[list(range(N_CORES))],
            ins=[cin.opt()],
            outs=[cout.opt()],
        )
    red = stat.tile([128, 2], F32, tag="red", name="red")
    nc.gpsimd.dma_start(out=red[:, :], in_=cout[:, :])

    inv_n = 1.0 / float(B * P)
    epsbn = consts.tile([128, 1], F32, tag="epsbn", name="epsbn")
    nc.vector.memset(epsbn, BN_EPS)
    me = stat.tile([128, 2], F32, tag="me", name="me")  # [mean, E[x^2]]
    nc.vector.tensor_scalar_mul(out=me, in0=red[:, :], scalar1=inv_n)
    msq = small.tile([128, 1], F32, tag="msq", name="msq")
    nc.vector.tensor_mul(out=msq, in0=me[:, 0:1], in1=me[:, 0:1])
    var = small.tile([128, 1], F32, tag="var", name="var")
    nc.vector.tensor_sub(out=var, in0=me[:, 1:2], in1=msq)
    invs = small.tile([128, 1], F32, tag="invs", name="invs")
    if USE_LNEXP:
        lt2 = small.tile([128, 1], F32, tag="lt2", name="lt2")
        nc.scalar.activation(out=lt2, in_=var, func=AF.Ln, bias=epsbn)
        nc.scalar.activation(out=invs, in_=lt2, func=AF.Exp, scale=-0.5)
    else:
        sdv = small.tile([128, 1], F32, tag="sdv", name="sdv")
        nc.scalar.activation(out=sdv, in_=var, func=AF.Sqrt, bias=epsbn)
        nc.vector.reciprocal(invs, sdv)
    scl = small.tile([128, 1], F32, tag="scl", name="scl")
    nc.vector.tensor_mul(out=scl, in0=bng, in1=invs)
    tmp = small.tile([128, 1], F32, tag="tmp", name="tmp")
    nc.vector.tensor_mul(out=tmp, in0=me[:, 0:1], in1=scl)
    shf = small.tile([128, 1], F32, tag="shf", name="shf")
    nc.vector.tensor_sub(out=shf, in0=bnb, in1=tmp)

    dq = [nc.gpsimd, nc.sync, nc.scalar]
    for pt in range(NPC):
        for b in range(BPC):
            seg = segs[b][:, pt, :]
            if (pt * BPC + b) % 4 != 3:
                nc.vector.tensor_scalar(out=seg, in0=seg, scalar1=scl, scalar2=shf,
                                        op0=ALU.mult, op1=ALU.add)
            else:
                nc.scalar.activation(out=seg, in_=seg, func=AF.Identity,
                                     scale=scl, bias=shf)
            dq[(pt * BPC + b) % 3].dma_start(out=io["out"][b, :, pt * 512:(pt + 1) * 512],
                                             in_=seg)


def _build():
    key = (SKIP_COLL, N_WARM_AR, USE_LNEXP)
    if key in _BUILD_CACHE:
        return _BUILD_CACHE[key]
    nc = bacc.Bacc("TRN2", target_bir_lowering=False, debug=False, num_devices=N_CORES)
    io = {
        "qT": nc.dram_tensor("qT", [BPC, NPC, 128, 4, 128], BF16, kind="ExternalInput").ap(),
        "kT": nc.dram_tensor("kT", [BPC, NPC, 128, 4, 128], BF16, kind="ExternalInput").ap(),
        "vT": nc.dram_tensor("vT", [BPC, NPC, 128, 4, 128], BF16, kind="ExternalInput").ap(),
        "resid": nc.dram_tensor("resid", [BPC, C, NPC, 512], BF16, kind="ExternalInput").ap(),
        "wq": nc.dram_tensor("wq", [NPC, 128, 4, D], BF16, kind="ExternalInput").ap(),
        "wk": nc.dram_tensor("wk", [NPC, 128, 4, D], BF16, kind="ExternalInput").ap(),
        "wv": nc.dram_tensor("wv", [NPC, 128, 4, D], BF16, kind="ExternalInput").ap(),
        "wfc": nc.dram_tensor("wfc", [NPC, 128, 4, 512], BF16, kind="ExternalInput").ap(),
        "colsum": nc.dram_tensor("colsum", [128, 4, NPC], BF16, kind="ExternalInput").ap(),
        "vsums": nc.dram_tensor("vsums", [C, 2 * NPC], F32, kind="ExternalInput").ap(),
        "bng": nc.dram_tensor("bng", [C, 1], F32, kind="ExternalInput").ap(),
        "bnb": nc.dram_tensor("bnb", [C, 1], F32, kind="ExternalInput").ap(),
        "out": nc.dram_tensor("out", [BPC, C, P], BF16, kind="ExternalOutput").ap(),
    }
    from contextlib import ExitStack
    with tile.TileContext(nc) as tc, ExitStack() as ctx:
        _emit(ctx, nc, tc, io)
    nc.compile()
    _BUILD_CACHE[key] = nc
    return nc


def _bf16(x):
    import ml_dtypes
    return np.ascontiguousarray(np.asarray(x, np.float32).astype(ml_dtypes.bfloat16))


def _pack_acts(xT):
    # [b, 4096, 128] -> [b, NPC, 128, 4, 128]  (pc-chunk, partition, j, c)
    b = xT.shape[0]
    return np.ascontiguousarray(
        xT.reshape(b, NPC, 4, 128, 128).transpose(0, 1, 3, 2, 4))


def _pack_w(w):
    # [4096, D] -> [NPC, 128, 4, D]
    return np.ascontiguousarray(w.reshape(NPC, 4, 128, -1).transpose(0, 2, 1, 3))


def kernel(v, k, q, w_qs, w_ks, w_vs, w_fc, ln_gamma, ln_beta, temperature,
           bn_gamma, bn_beta, **_ignored):
    v = np.asarray(v, np.float32)
    k = np.asarray(k, np.float32)
    q = np.asarray(q, np.float32)
    w_qs = np.asarray(w_qs, np.float32)
    w_ks = np.asarray(w_ks, np.float32)
    w_vs = np.asarray(w_vs, np.float32)
    w_fc = np.asarray(w_fc, np.float32)
    ln_gamma = np.asarray(ln_gamma, np.float32)
    ln_beta = np.asarray(ln_beta, np.float32)
    temp = float(np.asarray(temperature))
    bn_gamma = np.asarray(bn_gamma, np.float32)
    bn_beta = np.asarray(bn_beta, np.float32)

    qf = q.reshape(B, C, P)
    kf = k.reshape(B, C, P)
    vf = v.reshape(B, C, P)
    qT = _bf16(_pack_acts(qf.transpose(0, 2, 1)))
    kT = _bf16(_pack_acts(kf.transpose(0, 2, 1)))
    vT = _bf16(_pack_acts(vf.transpose(0, 2, 1)))
    wq = _bf16(_pack_w((w_qs / temp).T))
    wk = _bf16(_pack_w(w_ks.T))
    wv = _bf16(_pack_w(w_vs.T))
    # wfc packed as [pt, d_low, dc, p_in_pt]: wfcT_eff[dc*128+d_low, pt*512+p]
    wfcT_eff = (w_fc * ln_gamma[None, :]).T  # [D, P]
    wfc = _bf16(wfcT_eff.reshape(4, 128, NPC, 512).transpose(2, 1, 0, 3))
    # colsum[d_low, dc, pt] = sum_{p in pt} wfcT_eff[dc*128+d_low, pt*512+p]
    colsum = _bf16(wfcT_eff.reshape(4, 128, NPC, 512).sum(-1).transpose(1, 0, 2))
    bias_fc = (w_fc @ ln_beta).astype(np.float32)
    veff = vf + bias_fc[None, None, :]                      # [B, C, P] f32
    resid = _bf16(veff.reshape(B, C, NPC, 512))
    # per-(b,pt) channel sums of the (bf16-rounded) residual, f32 accumulated
    vsums_full = np.asarray(resid, np.float32).sum(-1)      # [B, C, NPC]
    bng = np.ascontiguousarray(bn_gamma.reshape(C, 1))
    bnb = np.ascontiguousarray(bn_beta.reshape(C, 1))

    nc = _build()
    in_maps = []
    for i in range(N_CORES):
        bs = slice(BPC * i, BPC * (i + 1))
        vsums = np.ascontiguousarray(
            vsums_full[bs].transpose(1, 0, 2).reshape(C, 2 * NPC))
        in_maps.append({
            "qT": qT[bs], "kT": kT[bs], "vT": vT[bs], "resid": resid[bs],
            "wq": wq, "wk": wk, "wv": wv, "wfc": wfc,
            "colsum": colsum, "vsums": vsums,
            "bng": bng, "bnb": bnb,
        })
    res = run_bass_kernel_spmd(nc, in_maps, core_ids=list(range(N_CORES)))
    global LAST_RESULTS
    LAST_RESULTS = res
    out = np.concatenate([np.asarray(res.results[i]["out"], np.float32)
                          for i in range(N_CORES)], axis=0)
    return out.reshape(B, C, HH, WW)
